# revision 6
# baseline (speedup 1.0000x reference)
"""AscendRejectionSampler — Trainium2 Bass kernel (8-core SPMD), v2.

kernel(**inputs) takes the full unsharded inputs and returns the full
[64, 9] int32 output.

Sharding: data-parallel over requests, 8 requests per core, balanced as
4 non-greedy (slots 0-3) + 4 greedy (slots 4-7).  Device program:

- tgt+drf uploaded as ONE [128, 32000] tensor (rows 0-63 target, 64-127
  draft) so dp/tp and the t/d segment reads are single multi-offset
  indirect gathers;
- target-probs big load split into 4 chunks with the chunk-max reduces
  interleaved on DVE;
- every cross-partition step uses DVE 32x32 block-transposes and tiny
  one-hot fp32 PE matmuls instead of DMA shuffles;
- approx reciprocal without NR refinement;
- all constants / masks / f32 scalars packed host-side into one int32
  tensor pkall [128, *].
"""
import sys
sys.path.insert(0, '/opt/trn_rl_repo')
import numpy as np
import concourse.bass as bass
import concourse.bacc as bacc
import concourse.tile as tile
from concourse import mybir
from concourse import bass_utils

f32 = mybir.dt.float32
i32 = mybir.dt.int32
u32 = mybir.dt.uint32
Alu = mybir.AluOpType
AX = mybir.AxisListType

R = 8
S = 8
SP1 = 9
V = 32000
ROWS = 64
NG = 4            # slots 0-3 non-greedy, 4-7 greedy
GROWS = 32        # greedy token rows (td rows 32..64)
QW = 8000         # quarter width
CW = 125          # chunk width
NCHQ = QW // CW   # 64 chunks per quarter
SEGW = 1000       # seg width (32 segs per row)
TLC = 2000        # target-load chunk width (per partition), x4

# pkall [128, PKALLW].  Three regions:
# cols 0..C64: per-token-partition data (rows 0-63)
P64_OFF = 0       # i32 [2]: tp offset p*V+tok, dp offset (64+p)*V+tok
P64_U = 2
P64_VREV = 3      # (j<nper)*(8-j)
P64_MSEL = 4      # [4]: p<32: (s == p//8)
P64_BASE = 8      # p<32: (p//8)*256 + 256
C64 = 9
# cols C64..C8: per-slot data (rows 0-7)
P8_TOK9 = C64         # [9] draft tokens as f32 (col 8 = 0)
P8_NPER = C64 + 9
P8_BON = C64 + 10
P8_OIN = C64 + 11     # [9]
P8_J9 = C64 + 20      # [9]
P8_V9 = C64 + 29      # [9] j < nper
P8_JEQ = C64 + 38     # [9] j == nper
P8_VREV8 = C64 + 47   # [8] (j<nper)*(8-j)
P8_ISG9 = C64 + 55    # [9] i32 is_greedy mask
P8_M2D = C64 + 64     # [4] (c == p)
C8 = C64 + 68
# cols C8..: per-128-partition constants
P128_W0 = C8          # i32: (32 + p//4)*256 + (p%4)*64
P128_QOFF = C8 + 1    # (p%4)*8000
P128_R8 = C8 + 2      # [8]: (r == (p%32)//4)
P128_REVQ = C8 + 10   # 4 - p%4
P128_LTGT = C8 + 11   # [8]: (m == 4 + p//32)
P128_SEGB = C8 + 19   # (p%32)*1000
P128_REVS = C8 + 20   # 32 - p%32
P128_LSEG = C8 + 21   # [8]: (m == p//32)
P128_SEGID = C8 + 29  # p%32
P128_LFR = C8 + 30    # [128], rows p<32: -32*(p == 8*(m//32))
P128_SEGID2 = C8 + 158   # (p//32)*256 + 256 + p%32   (ng soffs affine)
P128_SEGID2B = C8 + 159  # same + 2048                (draft-half row)
P128_R32 = C8 + 160      # [32]: (j//4 == (p%32)//4)
P128_QID = C8 + 192      # p%4
PKALLW = C8 + 193


def build_kernel_fast(n_devices=8):
    nc = bacc.Bacc("TRN2", target_bir_lowering=False, debug=False,
                   enable_asserts=False, num_devices=n_devices)
    td = nc.dram_tensor("td", [128, V], f32, kind="ExternalInput").ap()
    q4 = nc.dram_tensor("q4", [NG, V], f32, kind="ExternalInput").ap()
    pkall = nc.dram_tensor("pkall", [128, PKALLW], i32,
                           kind="ExternalInput").ap()
    out = nc.dram_tensor("out", [R, SP1], i32, kind="ExternalOutput").ap()
    with tile.TileContext(nc) as tc:
        _body(tc, nc, td, q4, pkall, out)
    nc.compile()
    return nc


def _body(tc, nc, td, q4, pkall, out):
    import contextlib
    ctx = contextlib.ExitStack()
    with ctx:
        small = ctx.enter_context(tc.tile_pool(name="small", bufs=1))
        big = ctx.enter_context(tc.tile_pool(name="big", bufs=1))
        psum = ctx.enter_context(tc.tile_pool(name="psum", bufs=1,
                                              space="PSUM"))

        V_ = nc.vector
        G_ = nc.gpsimd
        A_ = nc.scalar

        def ts(out_, in_, s1, op0, s2=None, op1=None):
            if op1 is None:
                return V_.tensor_scalar(out_, in_, s1, None, op0=op0)
            return V_.tensor_scalar(out_, in_, s1, s2, op0=op0, op1=op1)

        def tt(out_, a, b, op):
            return V_.tensor_tensor(out_, a, b, op=op)

        def stt(out_, in0, scalar, op0, in1, op1):
            return V_.scalar_tensor_tensor(out_, in0, scalar, in1, op0=op0,
                                           op1=op1)

        # ---------------- sync (HWDGE) loads, issue order = queue order
        pkt = small.tile([128, PKALLW], i32)
        nc.sync.dma_start(pkt[:], pkall[:, :])
        q4t = big.tile([128, SEGW], f32)
        nc.sync.dma_start(q4t[:], q4.rearrange("a (h c) -> (a h) c", c=SEGW))
        tgtv = td[GROWS:ROWS, :].rearrange("a (h c) -> (a h) c", c=QW)
        tl = []
        for k in range(4):
            t = big.tile([128, TLC], f32, name=f"tl{k}")
            nc.sync.dma_start(t[:], tgtv[:, k * TLC:(k + 1) * TLC])
            tl.append(t)

        # ---------------- views into the packed tile
        p8 = pkt[0:R, :]
        tokf9 = p8[:, P8_TOK9:P8_TOK9 + SP1].bitcast(f32)
        tokf8 = p8[:, P8_TOK9:P8_TOK9 + S].bitcast(f32)
        nperf = p8[:, P8_NPER:P8_NPER + 1].bitcast(f32)
        bonf = p8[:, P8_BON:P8_BON + 1].bitcast(f32)
        oinf9 = p8[:, P8_OIN:P8_OIN + SP1].bitcast(f32)
        j9f = p8[:, P8_J9:P8_J9 + SP1].bitcast(f32)
        validj9 = p8[:, P8_V9:P8_V9 + SP1].bitcast(f32)
        jeqn = p8[:, P8_JEQ:P8_JEQ + SP1].bitcast(f32)
        vrev8 = p8[:, P8_VREV8:P8_VREV8 + S].bitcast(f32)
        isg9 = p8[:, P8_ISG9:P8_ISG9 + SP1]
        m2diag = p8[:, P8_M2D:P8_M2D + NG].bitcast(f32)

        p64 = pkt[0:ROWS, :]
        offs2 = p64[:, P64_OFF:P64_OFF + 2]
        u64 = p64[:, P64_U:P64_U + 1].bitcast(f32)
        vrev64 = p64[:, P64_VREV:P64_VREV + 1].bitcast(f32)
        msel = p64[:, P64_MSEL:P64_MSEL + NG].bitcast(f32)
        basecol = p64[:, P64_BASE:P64_BASE + 1].bitcast(f32)

        woffs0 = pkt[:, P128_W0:P128_W0 + 1]
        qoffs = pkt[:, P128_QOFF:P128_QOFF + 1].bitcast(f32)
        r8m = pkt[:, P128_R8:P128_R8 + 8].bitcast(f32)
        revq = pkt[:, P128_REVQ:P128_REVQ + 1].bitcast(f32)
        lhsTtgt = pkt[:, P128_LTGT:P128_LTGT + 8].bitcast(f32)
        segbase = pkt[:, P128_SEGB:P128_SEGB + 1].bitcast(f32)
        revseg = pkt[:, P128_REVS:P128_REVS + 1].bitcast(f32)
        lhsTseg = pkt[:, P128_LSEG:P128_LSEG + 8].bitcast(f32)
        segid = pkt[:, P128_SEGID:P128_SEGID + 1].bitcast(f32)
        lhsTfr = pkt[0:32, P128_LFR:P128_LFR + 128].bitcast(f32)
        segid2 = pkt[:, P128_SEGID2:P128_SEGID2 + 1].bitcast(f32)
        segid2b = pkt[:, P128_SEGID2B:P128_SEGID2B + 1].bitcast(f32)
        r32m = pkt[:, P128_R32:P128_R32 + 32].bitcast(f32)
        qidf = pkt[:, P128_QID:P128_QID + 1].bitcast(f32)

        # ---------------- gpsimd: tp/dp gathers
        tdflat = td.rearrange("a b -> (a b)").unsqueeze(-1)
        tp64 = small.tile([ROWS, 1], f32)
        G_.indirect_dma_start(
            out=tp64[:], out_offset=None, in_=tdflat,
            in_offset=bass.IndirectOffsetOnAxis(ap=offs2[:, 0:1], axis=0))
        dp64 = small.tile([ROWS, 1], f32)
        G_.indirect_dma_start(
            out=dp64[:], out_offset=None, in_=tdflat,
            in_offset=bass.IndirectOffsetOnAxis(ap=offs2[:, 1:2], axis=0))

        # ---------------- early memsets (DVE)
        am9 = small.tile([R, SP1], f32)
        V_.memset(am9[:, S:SP1], 0.0)
        inmax8 = small.tile([128, 8], f32)
        V_.memset(inmax8[:, 1:8], -3.0e38)
        rv8 = small.tile([128, 8], f32)
        V_.memset(rv8[:, 4:8], -1.0)

        # ---------------- DVE: accept logic in [64,1] layout
        # accept <=> dp*u <= tp (dp > 0 always for softmax inputs)
        c2 = small.tile([ROWS, 1], f32)
        stt(c2[:], dp64[:], u64, Alu.mult, tp64[:], Alu.is_le)
        rejrev = small.tile([ROWS, 1], f32)
        stt(rejrev[:], c2[:], 0.0, Alu.is_equal, vrev64, Alu.mult)

        # ---------------- first_rej per slot via block transpose
        b64 = small.tile([ROWS, 32], f32)
        V_.tensor_copy(out=b64[:], in_=rejrev[:].to_broadcast([ROWS, 32]))
        t64 = small.tile([ROWS, 32], f32)
        V_.transpose(out=t64[:], in_=b64[:])
        mrev_all = small.tile([ROWS, NG], f32)
        V_.tensor_reduce(mrev_all[:],
                         t64[:].rearrange("p (a b) -> p a b", b=8),
                         axis=AX.X, op=Alu.max)
        scr64 = small.tile([ROWS, NG], f32)
        tt(scr64[:], mrev_all[:], msel, Alu.mult)
        mrevsel = small.tile([ROWS, 1], f32)
        V_.tensor_reduce(mrevsel[:], scr64[:], axis=AX.X, op=Alu.max)

        # -32*mrev(slot) distributed to all 128 partitions via one-hot
        # matmul (scale folded into lhsT); affine finish + i32 cast on ACT
        base128 = psum.tile([128, 1], f32)
        nc.tensor.matmul(base128[:], lhsTfr, mrevsel[0:32, 0:1])
        soffs2 = small.tile([128, 2], i32)
        A_.activation(soffs2[:, 0:1], base128[:],
                      mybir.ActivationFunctionType.Identity, bias=segid2)
        A_.activation(soffs2[:, 1:2], base128[:],
                      mybir.ActivationFunctionType.Identity, bias=segid2b)

        # ---------------- gpsimd: t/d seg gathers + diff
        tdv = td.rearrange("a (b c) -> (a b) c", c=SEGW)
        t_seg = big.tile([128, SEGW], f32)
        G_.indirect_dma_start(
            out=t_seg[:], out_offset=None, in_=tdv,
            in_offset=bass.IndirectOffsetOnAxis(ap=soffs2[:, 0:1], axis=0))
        d_seg = big.tile([128, SEGW], f32)
        G_.indirect_dma_start(
            out=d_seg[:], out_offset=None, in_=tdv,
            in_offset=bass.IndirectOffsetOnAxis(ap=soffs2[:, 1:2], axis=0))


        # ---------------- DVE: reciprocal of q (approx, no NR)
        rq = big.tile([128, SEGW], f32)
        V_.reciprocal_approx_fast(out=rq[:], in_=q4t[:])

        # ---------------- DVE: all 4 chunk reduces
        cmax = small.tile([128, NCHQ], f32)
        NCC = TLC // CW  # 16 chunks per tl chunk
        for k in range(4):
            V_.tensor_reduce(cmax[:, k * NCC:(k + 1) * NCC],
                             tl[k][:].rearrange("p (b c) -> p b c", c=CW),
                             axis=AX.X, op=Alu.max)

        # ---------------- DVE: ratio scan
        diff = big.tile([128, SEGW], f32)
        tt(diff[:], t_seg[:], d_seg[:], Alu.subtract)
        ratio = big.tile([128, SEGW], f32)
        tt(ratio[:], diff[:], rq[:], Alu.mult)
        V_.tensor_reduce(inmax8[:, 0:1], ratio[:], axis=AX.X, op=Alu.max)
        is8 = small.tile([128, 8], u32)
        V_.max_index(out=is8[:], in_max=inmax8[:], in_values=ratio[:])
        i_f = small.tile([128, 1], f32)
        A_.copy(out=i_f[:], in_=is8[:, 0:1])

        # ---------------- target argmax: chunk pick + window gather
        c8 = small.tile([128, 8], f32)
        V_.max(out=c8[:], in_=cmax[:])
        ci8 = small.tile([128, 8], u32)
        V_.max_index(out=ci8[:], in_max=c8[:], in_values=cmax[:])
        ci_i = small.tile([128, 1], i32)
        A_.copy(out=ci_i[:], in_=ci8[:, 0:1])
        cif = small.tile([128, 1], f32)
        A_.copy(out=cif[:], in_=ci_i[:])
        woffs = small.tile([128, 1], i32)
        tt(woffs[:], woffs0, ci_i[:], Alu.add)
        wt = small.tile([128, CW], f32)
        G_.indirect_dma_start(
            out=wt[:], out_offset=None,
            in_=td.rearrange("a (b c) -> (a b) c", c=CW),
            in_offset=bass.IndirectOffsetOnAxis(ap=woffs[:], axis=0))

        # ---------------- seg argmax finish (band == slot)
        # transpose gives every partition its band's 32 seg-maxes; the
        # band argmax (first occurrence) is the winning seg id
        mseg = inmax8[:, 0:1]
        bs = small.tile([128, 32], f32)
        V_.tensor_copy(out=bs[:], in_=mseg.to_broadcast([128, 32]))
        ts_t = small.tile([128, 32], f32)
        V_.transpose(out=ts_t[:], in_=bs[:])
        sx8 = small.tile([128, 8], f32)
        V_.max(out=sx8[:], in_=ts_t[:])
        si8 = small.tile([128, 8], u32)
        V_.max_index(out=si8[:], in_max=sx8[:], in_values=ts_t[:])
        s_f = small.tile([128, 1], f32)
        A_.copy(out=s_f[:], in_=si8[:, 0:1])
        iswin = small.tile([128, 1], f32)
        tt(iswin[:], segid, s_f[:], Alu.is_equal)
        recval = small.tile([128, 1], f32)
        ts(recval[:], i_f[:], segbase, Alu.add)
        rec_p = small.tile([128, 1], f32)
        tt(rec_p[:], recval[:], iswin[:], Alu.mult)
        rec8p = psum.tile([R, 1], f32)
        nc.tensor.matmul(rec8p[:], lhsTseg, rec_p[:])
        rec_sb = small.tile([R, 1], f32)
        A_.copy(out=rec_sb[:], in_=rec8p[:])

        # ---------------- cross-quarter winner while the window gather flies
        # transpose + own-row mask + strided reduce -> the row's 4 quarter
        # maxes per partition; argmax (first occurrence) = winning quarter
        m128 = c8[:, 0:1]
        bt = small.tile([128, 32], f32)
        V_.tensor_copy(out=bt[:], in_=m128.to_broadcast([128, 32]))
        tt_t = small.tile([128, 32], f32)
        V_.transpose(out=tt_t[:], in_=bt[:])
        prod = small.tile([128, 32], f32)
        tt(prod[:], tt_t[:], r32m, Alu.mult)
        V_.tensor_reduce(rv8[:, 0:4],
                         prod[:].rearrange("p (a b) -> p b a", b=4),
                         axis=AX.X, op=Alu.max)
        qx8 = small.tile([128, 8], f32)
        V_.max(out=qx8[:], in_=rv8[:])
        qi8 = small.tile([128, 8], u32)
        V_.max_index(out=qi8[:], in_max=qx8[:], in_values=rv8[:])
        q_f = small.tile([128, 1], f32)
        A_.copy(out=q_f[:], in_=qi8[:, 0:1])
        iswint = small.tile([128, 1], f32)
        tt(iswint[:], qidf, q_f[:], Alu.is_equal)

        # window finish
        w8 = small.tile([128, 8], f32)
        V_.max(out=w8[:], in_=wt[:])
        wi8 = small.tile([128, 8], u32)
        V_.max_index(out=wi8[:], in_max=w8[:], in_values=wt[:])
        wif = small.tile([128, 1], f32)
        A_.copy(out=wif[:], in_=wi8[:, 0:1])
        qam = small.tile([128, 1], f32)
        ts(qam[:], cif[:], float(CW), Alu.mult, qoffs, Alu.add)
        tt(qam[:], qam[:], wif[:], Alu.add)
        amw = small.tile([128, 1], f32)
        tt(amw[:], qam[:], iswint[:], Alu.mult)
        rhs_tgt = small.tile([128, 8], f32)
        tt(rhs_tgt[:], amw[:].to_broadcast([128, 8]), r8m, Alu.mult)
        tgtamp = psum.tile([R, S], f32)
        nc.tensor.matmul(tgtamp[:], lhsTtgt, rhs_tgt[:])
        A_.copy(out=am9[:, 0:S], in_=tgtamp[:])

        # ---------------- greedy logic [8,8]
        mism = small.tile([R, S], f32)
        tt(mism[:], tokf8, tgtamp[:], Alu.not_equal)
        mmrev = small.tile([R, S], f32)
        tt(mmrev[:], mism[:], vrev8, Alu.mult)
        mm_max = small.tile([R, 1], f32)
        V_.tensor_reduce(mm_max[:], mmrev[:], axis=AX.X, op=Alu.max)
        first_mm = small.tile([R, 1], f32)
        ts(first_mm[:], mm_max[:], -1.0, Alu.mult, float(S), Alu.add)
        copy_len = small.tile([R, 1], f32)
        ts(copy_len[:], first_mm[:], 1.0, Alu.add)
        tt(copy_len[:], copy_len[:], nperf, Alu.min)

        # first_rej per slot for the [8,*] assembly
        scr8 = small.tile([R, NG], f32)
        tt(scr8[:], mrev_all[0:R, :], m2diag, Alu.mult)
        mrevdiag = small.tile([R, 1], f32)
        V_.tensor_reduce(mrevdiag[:], scr8[:], axis=AX.X, op=Alu.max)
        fr8 = small.tile([R, 1], f32)
        ts(fr8[:], mrevdiag[:], -1.0, Alu.mult, float(S), Alu.add)

        # ---------------- output assembly
        on = small.tile([R, SP1], f32)
        A_.copy(out=on[:], in_=oinf9)
        og = small.tile([R, SP1], f32)
        A_.copy(out=og[:], in_=oinf9)
        dmask = small.tile([R, SP1], i32)
        stt(dmask[:], j9f, fr8[:, 0:1], Alu.is_lt, validj9, Alu.mult)
        V_.copy_predicated(on[:], dmask[:], tokf9)
        rmask = small.tile([R, SP1], i32)
        stt(rmask[:], j9f, fr8[:, 0:1], Alu.is_equal, validj9, Alu.mult)
        V_.copy_predicated(on[:], rmask[:], rec_sb[:].to_broadcast([R, SP1]))
        bmn = small.tile([R, SP1], i32)
        stt(bmn[:], fr8[:].to_broadcast([R, SP1]), nperf[:, 0:1], Alu.is_ge,
            jeqn, Alu.mult)
        V_.copy_predicated(on[:], bmn[:], bonf.to_broadcast([R, SP1]))
        clm = small.tile([R, SP1], i32)
        ts(clm[:], j9f, copy_len[:, 0:1], Alu.is_lt)
        V_.copy_predicated(og[:], clm[:], am9[:])
        bmg = small.tile([R, SP1], i32)
        stt(bmg[:], first_mm[:].to_broadcast([R, SP1]), nperf[:, 0:1],
            Alu.is_ge, jeqn, Alu.mult)
        V_.copy_predicated(og[:], bmg[:], bonf.to_broadcast([R, SP1]))
        V_.copy_predicated(on[:], isg9, og[:])
        outi = small.tile([R, SP1], i32)
        V_.tensor_copy(out=outi[:], in_=on[:])
        nc.sync.dma_start(out[:, :], outi[:])


# ---------------- host-side packing ----------------

def plan_permutation(inputs, n_cores=8):
    """Slot assignment: 4 non-greedy (slots 0-3) + 4 greedy (slots 4-7)."""
    isg = np.asarray(inputs["is_greedy"]).astype(bool)
    B = isg.shape[0]
    if B != ROWS or B // n_cores != R:
        return None
    g = np.where(isg)[0]
    n = np.where(~isg)[0]
    if len(g) != B // 2:
        return None
    perm = np.empty(B, np.int64)
    for c in range(n_cores):
        perm[c * R:c * R + NG] = n[c * NG:(c + 1) * NG]
        perm[c * R + NG:(c + 1) * R] = g[c * NG:(c + 1) * NG]
    return perm


def _f32bits(x):
    return np.asarray(x, dtype=np.float32).view(np.int32)


def _pk_static():
    """The per-128-partition constant columns (identical for all cores)."""
    p128 = np.arange(128)
    m8 = np.arange(8)[None, :]
    pk = np.zeros((128, PKALLW), np.int32)
    pk[:, P128_W0] = (GROWS + p128 // 4) * (V // CW) + (p128 % 4) * NCHQ
    pk[:, P128_QOFF] = _f32bits((p128 % 4) * QW)
    pk[:, P128_R8:P128_R8 + 8] = _f32bits((m8 == ((p128 % 32) // 4)[:, None]))
    pk[:, P128_REVQ] = _f32bits(4 - p128 % 4)
    pk[:, P128_LTGT:P128_LTGT + 8] = _f32bits((m8 == (4 + p128 // 32)[:, None]))
    pk[:, P128_SEGB] = _f32bits((p128 % 32) * SEGW)
    pk[:, P128_REVS] = _f32bits(32 - p128 % 32)
    pk[:, P128_LSEG:P128_LSEG + 8] = _f32bits((m8 == (p128 // 32)[:, None]))
    pk[:, P128_SEGID] = _f32bits(p128 % 32)
    kk = np.arange(32)[:, None]
    mm = np.arange(128)[None, :]
    pk[0:32, P128_LFR:P128_LFR + 128] = _f32bits(
        -32.0 * (kk == 8 * (mm // 32)))
    pk[:, P128_SEGID2] = _f32bits((p128 // 32) * 256 + 256 + p128 % 32)
    pk[:, P128_SEGID2B] = _f32bits((p128 // 32) * 256 + 256 + p128 % 32
                                   + 2048)
    j32 = np.arange(32)[None, :]
    pk[:, P128_R32:P128_R32 + 32] = _f32bits(
        (j32 // 4) == ((p128 % 32) // 4)[:, None])
    pk[:, P128_QID] = _f32bits(p128 % 4)
    # [64,*] static region
    mselm = np.zeros((ROWS, NG), np.float32)
    mselm[0:32] = (np.arange(NG)[None, :] == (np.arange(32) // 8)[:, None])
    pk[0:ROWS, P64_MSEL:P64_MSEL + NG] = _f32bits(mselm)
    basec = np.zeros(ROWS, np.float32)
    basec[0:32] = (np.arange(32) // 8) * 256 + 256
    pk[0:ROWS, P64_BASE] = _f32bits(basec)
    # [8,*] static region
    j9 = np.arange(SP1)[None, :]
    pk[0:R, P8_J9:P8_J9 + SP1] = _f32bits(np.broadcast_to(j9, (R, SP1)))
    pk[0:R, P8_M2D:P8_M2D + NG] = _f32bits(
        np.arange(NG)[None, :] == np.arange(R)[:, None])
    return pk


_PK_STATIC = None


def shard_inputs_fast(inputs, perm, n_cores=8):
    global _PK_STATIC
    if _PK_STATIC is None:
        _PK_STATIC = _pk_static()
    cu = inputs["cu_num_draft_tokens"].astype(np.int64)
    N = inputs["draft_token_ids"].shape[0]
    n_per = np.diff(np.concatenate([[0], cu]))
    start = cu - n_per
    gidx = np.clip(start[:, None] + np.arange(S)[None, :], 0, N - 1)  # [B,S]
    jj = np.arange(S)[None, :]
    j9 = np.arange(SP1)[None, :]
    p64 = np.arange(ROWS)

    in_maps = []
    for c in range(n_cores):
        reqs = perm[c * R:(c + 1) * R]
        g = gidx[reqs]                     # [8, 8] global token-row ids
        rows = g.reshape(-1)
        npr = n_per[reqs][:, None]         # [8, 1]
        tokc = inputs["draft_token_ids"][g].astype(np.int64)   # [8, 8]

        pk = _PK_STATIC.copy()
        tok9 = np.zeros((R, SP1), np.float32)
        tok9[:, 0:S] = tokc
        pk[0:R, P8_TOK9:P8_TOK9 + SP1] = _f32bits(tok9)
        pk[0:R, P8_NPER] = _f32bits(n_per[reqs])
        pk[0:R, P8_BON] = _f32bits(inputs["bonus_token_ids"][reqs])
        pk[0:R, P8_OIN:P8_OIN + SP1] = _f32bits(
            inputs["output_token_ids"][reqs])
        pk[0:R, P8_V9:P8_V9 + SP1] = _f32bits(j9 < npr)
        pk[0:R, P8_JEQ:P8_JEQ + SP1] = _f32bits(j9 == npr)
        pk[0:R, P8_VREV8:P8_VREV8 + S] = _f32bits((jj < npr) * (S - jj))
        pk[0:R, P8_ISG9:P8_ISG9 + SP1] = (
            inputs["is_greedy"][reqs].astype(np.int32)[:, None]
            * np.ones((1, SP1), np.int32))

        flat_tok = tokc.reshape(-1)
        pk[0:ROWS, P64_OFF] = (p64 * V + flat_tok).astype(np.int32)
        pk[0:ROWS, P64_OFF + 1] = ((p64 + ROWS) * V + flat_tok).astype(np.int32)
        pk[0:ROWS, P64_U] = _f32bits(inputs["uniform_probs"][g].reshape(-1))
        vrev = ((jj < npr) * (S - jj)).astype(np.float32)
        pk[0:ROWS, P64_VREV] = _f32bits(vrev.reshape(-1))

        tdc = np.empty((128, V), np.float32)
        tdc[0:ROWS] = inputs["target_probs"][rows]
        tdc[ROWS:128] = inputs["draft_probs"][rows]
        in_maps.append(dict(
            td=tdc,
            q4=np.ascontiguousarray(inputs["q"][reqs[0:NG]],
                                    dtype=np.float32),
            pkall=pk,
        ))
    return in_maps


def assemble_outputs_fast(results, perm):
    raw = np.concatenate([r["out"] for r in results], axis=0).astype(np.int32)
    out = np.empty_like(raw)
    out[perm] = raw
    return out


from concourse.dve_ops import RECIPROCAL_APPROX_NR
from concourse.tile import add_dep_helper

G_TW = 8000
G_VT = 2
G_CW = 125
G_NCH = 16000 // G_CW
G_NWIN = V // G_CW
G_SEGW = 2000
G_HALF = 16000
G_C_TOK, G_C_U, G_C_NPER, G_C_LST, G_C_ISG, G_C_BON, G_C_OIN = 0, 8, 16, 17, 18, 19, 20
G_PKW = 29



def build_kernel_gen(n_devices=8):
    nc = bacc.Bacc("TRN2", target_bir_lowering=False, debug=False,
                   enable_asserts=True, num_devices=n_devices)
    tgt = nc.dram_tensor("tgt", [ROWS, V], f32, kind="ExternalInput").ap()
    drf = nc.dram_tensor("drf", [ROWS, V], f32, kind="ExternalInput").ap()
    q = nc.dram_tensor("q", [R, V], f32, kind="ExternalInput").ap()
    pk = nc.dram_tensor("pk", [R, G_PKW], i32, kind="ExternalInput").ap()
    out = nc.dram_tensor("out", [R, SP1], i32, kind="ExternalOutput").ap()
    import os as _os
    dbg = None
    if _os.environ.get("RSK_DBG"):
        dbg = nc.dram_tensor("dbg", [R, 4 * S], f32, kind="ExternalOutput").ap()
    with tile.TileContext(nc) as tc:
        _body_gen(tc, nc, tgt, drf, q, pk, out, dbg)
    nc.compile()
    return nc


def _body_gen(tc, nc, tgt, drf, q, pk, out, dbg=None):
    import contextlib
    ctx = contextlib.ExitStack()
    with ctx:
        small = ctx.enter_context(tc.tile_pool(name="small", bufs=1))
        stream = ctx.enter_context(tc.tile_pool(name="stream", bufs=1))
        segp = ctx.enter_context(tc.tile_pool(name="segp", bufs=1))

        V_ = nc.vector
        G_ = nc.gpsimd

        def ts(out_, in_, s1, op0, s2=None, op1=None):
            if op1 is None:
                V_.tensor_scalar(out_, in_, s1, None, op0=op0)
            else:
                V_.tensor_scalar(out_, in_, s1, s2, op0=op0, op1=op1)

        def tt(out_, a, b, op):
            V_.tensor_tensor(out_, a, b, op=op)

        def stt(out_, in0, scalar, op0, in1, op1):
            V_.scalar_tensor_tensor(out_, in0, scalar, in1, op0=op0, op1=op1)

        def acast(dst, src):
            nc.scalar.copy(out=dst, in_=src)

        # ---- sync ring: pk, q, tileA (the rest of sync is the final store)
        pkt = small.tile([R, G_PKW], i32)
        nc.sync.dma_start(pkt[:], pk[:, :])
        q_seg = segp.tile([128, G_SEGW], f32)
        nc.sync.dma_start(q_seg[:], q[:, :])
        tgt_v = tgt.rearrange("a (h c) -> (a h) c", c=G_HALF)  # [128, 16000]
        tlA = stream.tile([128, G_TW], f32, tag="tlA", name="tlA")
        nc.sync.dma_start(tlA[:], tgt_v[:, 0:G_TW])

        tok88i = pkt[:, G_C_TOK:G_C_TOK + 8]
        u88 = pkt[:, G_C_U:G_C_U + 8].bitcast(f32)
        nperi = pkt[:, G_C_NPER:G_C_NPER + 1]
        lsti = pkt[:, G_C_LST:G_C_LST + 1]
        isgi = pkt[:, G_C_ISG:G_C_ISG + 1]
        boni = pkt[:, G_C_BON:G_C_BON + 1]
        oin_i = pkt[:, G_C_OIN:G_C_OIN + SP1]

        # ---- gpsimd ring: iotas, tileB, then the gather chains
        j8i = small.tile([R, S], i32)
        G_.iota(j8i[:], pattern=[[1, S]], base=0, channel_multiplier=0)
        j9i = small.tile([R, SP1], i32)
        G_.iota(j9i[:], pattern=[[1, SP1]], base=0, channel_multiplier=0)
        rev8i = small.tile([R, S], i32)   # 8 - j
        G_.iota(rev8i[:], pattern=[[-1, S]], base=S, channel_multiplier=0)
        iota16 = small.tile([R, 16], i32)
        G_.iota(iota16[:], pattern=[[1, 16]], base=0, channel_multiplier=0)
        qoff0 = small.tile([R, 1], i32)   # r * G_NWIN
        G_.iota(qoff0[:], pattern=[[0, 1]], base=0, channel_multiplier=G_NWIN)
        woffs0 = small.tile([128, 1], i32)  # p * G_NCH
        G_.iota(woffs0[:], pattern=[[0, 1]], base=0, channel_multiplier=G_NCH)
        tlB = stream.tile([128, G_TW], f32, tag="tlB", name="tlB")
        G_.dma_start(tlB[:], tgt_v[:, G_TW:2 * G_TW])

        # ---- ACT casts of smalls
        tokf = small.tile([R, S], f32)
        acast(tokf[:], tok88i)
        nperf = small.tile([R, 1], f32)
        acast(nperf[:], nperi)
        lstf = small.tile([R, 1], f32)
        acast(lstf[:], lsti)
        isgf = small.tile([R, 1], f32)
        acast(isgf[:], isgi)
        bonf = small.tile([R, 1], f32)
        acast(bonf[:], boni)
        j8f = small.tile([R, S], f32)
        acast(j8f[:], j8i[:])
        j9f = small.tile([R, SP1], f32)
        acast(j9f[:], j9i[:])
        rev8f = small.tile([R, S], f32)
        acast(rev8f[:], rev8i[:])

        # ---- dp/tp gather offsets (DVE, f32-exact) -> [64,1]
        lr88f = small.tile([R, S], f32)
        V_.tensor_scalar(lr88f[:], j8f[:], lstf[:, 0:1], None, op0=Alu.add)
        ts(lr88f[:], lr88f[:], float(ROWS - 1), Alu.min, 0.0, Alu.max)
        offs88f = small.tile([R, S], f32)
        ts(offs88f[:], lr88f[:], float(V), Alu.mult)
        tt(offs88f[:], offs88f[:], tokf[:], Alu.add)
        offs88 = small.tile([R, S], i32)
        V_.tensor_copy(out=offs88[:], in_=offs88f[:])
        offs64 = small.tile([ROWS, 1], i32)
        G_.dma_start(offs64[:], offs88[:])
        dp64 = small.tile([ROWS, 1], f32)
        G_.indirect_dma_start(
            out=dp64[:], out_offset=None,
            in_=drf.rearrange("a b -> (a b)").unsqueeze(-1),
            in_offset=bass.IndirectOffsetOnAxis(ap=offs64[:], axis=0))
        tp64 = small.tile([ROWS, 1], f32)
        G_.indirect_dma_start(
            out=tp64[:], out_offset=None,
            in_=tgt.rearrange("a b -> (a b)").unsqueeze(-1),
            in_offset=bass.IndirectOffsetOnAxis(ap=offs64[:], axis=0))
        dp = small.tile([R, S], f32)
        G_.dma_start(dp[:], dp64[:])
        tp = small.tile([R, S], f32)
        G_.dma_start(tp[:], tp64[:])

        # ---- reciprocal of q (approx fast + one Newton step), early on DVE
        y0 = segp.tile([128, G_SEGW], f32)
        V_.reciprocal_approx_fast(out=y0[:], in_=q_seg[:])
        rq = y0
        V_._custom_dve(RECIPROCAL_APPROX_NR, out=rq[:], in0=q_seg[:],
                       in1=y0[:], s0=2.0)

        if dbg is not None:
            dbt = small.tile([R, 4 * S], f32)
            V_.tensor_copy(out=dbt[:, 0:S], in_=dp[:])
            V_.tensor_copy(out=dbt[:, S:2*S], in_=tp[:])
            V_.tensor_copy(out=dbt[:, 2*S:3*S], in_=u88)
            V_.tensor_copy(out=dbt[:, 3*S:4*S], in_=offs88f[:])
            nc.sync.dma_start(dbg[:, :], dbt[:])

        # ---- accept/reject logic (DVE smalls)
        rdp = small.tile([R, S], f32)
        V_.reciprocal(rdp[:], dp[:])
        rt = small.tile([R, S], f32)
        tt(rt[:], tp[:], rdp[:], Alu.mult)
        c2 = small.tile([R, S], f32)
        tt(c2[:], rt[:], u88, Alu.is_ge)
        acc = small.tile([R, S], f32)
        stt(acc[:], dp[:], 0.0, Alu.is_gt, c2[:], Alu.mult)
        validj = small.tile([R, S], f32)
        ts(validj[:], j8f[:], nperf[:, 0:1], Alu.is_lt)
        rej = small.tile([R, S], f32)
        stt(rej[:], acc[:], 0.0, Alu.is_equal, validj[:], Alu.mult)
        rejrev = small.tile([R, S], f32)
        tt(rejrev[:], rej[:], rev8f[:], Alu.mult)
        mrev = small.tile([R, 1], f32)
        V_.tensor_reduce(mrev[:], rejrev[:], axis=AX.X, op=Alu.max)
        first_rej = small.tile([R, 1], f32)   # 8 - mrev (8 when none)
        ts(first_rej[:], mrev[:], -1.0, Alu.mult, float(S), Alu.add)

        # ---- nstar + seg offsets
        nsf = small.tile([R, 1], f32)
        tt(nsf[:], lstf[:], first_rej[:], Alu.add)
        ts(nsf[:], nsf[:], float(ROWS - 1), Alu.min, 0.0, Alu.max)
        nstar_i = small.tile([R, 1], i32)
        acast(nstar_i[:], nsf[:])
        nst16f = small.tile([R, 1], f32)
        ts(nst16f[:], nsf[:], 16.0, Alu.mult)
        iota16f = small.tile([R, 16], f32)
        acast(iota16f[:], iota16[:])
        soff16f = small.tile([R, 16], f32)
        V_.tensor_scalar(soff16f[:], iota16f[:], nst16f[:, 0:1], None, op0=Alu.add)
        soff16 = small.tile([R, 16], i32)
        V_.tensor_copy(out=soff16[:], in_=soff16f[:])
        soffs = small.tile([128, 1], i32)
        G_.dma_start(soffs[:], soff16[:])
        t_seg = segp.tile([128, G_SEGW], f32)
        G_.indirect_dma_start(
            out=t_seg[:], out_offset=None,
            in_=tgt.rearrange("a (b c) -> (a b) c", c=G_SEGW),
            in_offset=bass.IndirectOffsetOnAxis(ap=soffs[:], axis=0))
        d_seg = segp.tile([128, G_SEGW], f32)
        G_.indirect_dma_start(
            out=d_seg[:], out_offset=None,
            in_=drf.rearrange("a (b c) -> (a b) c", c=G_SEGW),
            in_offset=bass.IndirectOffsetOnAxis(ap=soffs[:], axis=0))

        # ---- DVE heavy: MAXB, then ratio sub/mul/reduce, then MAXA
        cmax = small.tile([128, G_NCH], f32)
        V_.tensor_reduce(
            cmax[:, G_TW // G_CW:2 * (G_TW // G_CW)],
            tlB[:].rearrange("p (b c) -> p b c", c=G_CW),
            axis=AX.X, op=Alu.max)
        diff = t_seg
        tt(diff[:], t_seg[:], d_seg[:], Alu.subtract)
        ratio = d_seg
        tt(ratio[:], diff[:], rq[:], Alu.mult)
        rcm = small.tile([128, 16], f32)
        V_.tensor_reduce(rcm[:], ratio[:].rearrange("p (b c) -> p b c", c=G_CW),
                         axis=AX.X, op=Alu.max)
        rcm_req = small.tile([R, 16 * 16], f32)
        G_.dma_start(rcm_req[:], rcm[:])
        V_.tensor_reduce(
            cmax[:, 0:G_TW // G_CW],
            tlA[:].rearrange("p (b c) -> p b c", c=G_CW),
            axis=AX.X, op=Alu.max)

        # ---- ratio argmax finish
        f8 = small.tile([R, 8], f32)
        V_.max(out=f8[:], in_=rcm_req[:])
        fi8 = small.tile([R, 8], u32)
        V_.max_index(out=fi8[:], in_max=f8[:], in_values=rcm_req[:])
        fc_i = small.tile([R, 1], i32)
        acast(fc_i[:], fi8[:, 0:1])
        fcf = small.tile([R, 1], f32)
        acast(fcf[:], fc_i[:])
        woff_t = small.tile([R, 1], i32)
        ts(woff_t[:], nstar_i[:], G_NWIN, Alu.mult)
        tt(woff_t[:], woff_t[:], fc_i[:], Alu.add)
        qoff = small.tile([R, 1], i32)
        tt(qoff[:], qoff0[:], fc_i[:], Alu.add)
        t_win = small.tile([R, G_CW], f32)
        G_.indirect_dma_start(
            out=t_win[:], out_offset=None,
            in_=tgt.rearrange("a (b c) -> (a b) c", c=G_CW),
            in_offset=bass.IndirectOffsetOnAxis(ap=woff_t[:], axis=0))
        d_win = small.tile([R, G_CW], f32)
        G_.indirect_dma_start(
            out=d_win[:], out_offset=None,
            in_=drf.rearrange("a (b c) -> (a b) c", c=G_CW),
            in_offset=bass.IndirectOffsetOnAxis(ap=woff_t[:], axis=0))
        q_win = small.tile([R, G_CW], f32)
        G_.indirect_dma_start(
            out=q_win[:], out_offset=None,
            in_=q.rearrange("a (b c) -> (a b) c", c=G_CW),
            in_offset=bass.IndirectOffsetOnAxis(ap=qoff[:], axis=0))

        # ---- target argmax finish (DVE while window gathers fly)
        hm = small.tile([128, 1], f32)
        V_.tensor_reduce(hm[:], cmax[:], axis=AX.X, op=Alu.max)
        c8 = small.tile([128, 8], f32)
        V_.max(out=c8[:], in_=cmax[:])
        ci8 = small.tile([128, 8], u32)
        V_.max_index(out=ci8[:], in_max=c8[:], in_values=cmax[:])
        ci_i = small.tile([128, 1], i32)
        acast(ci_i[:], ci8[:, 0:1])
        cif = small.tile([128, 1], f32)
        acast(cif[:], ci_i[:])
        woffs = small.tile([128, 1], i32)
        tt(woffs[:], woffs0[:], ci_i[:], Alu.add)
        wt = small.tile([128, G_CW], f32)
        G_.indirect_dma_start(
            out=wt[:], out_offset=None,
            in_=tgt.rearrange("a (b c) -> (a b) c", c=G_CW),
            in_offset=bass.IndirectOffsetOnAxis(ap=woffs[:], axis=0))

        # ---- ratio window recompute (same instruction kinds => same bits)
        y0w = small.tile([R, G_CW], f32)
        y0ws = small.tile([R, G_CW], f32)
        V_.reciprocal_approx_fast(out=y0ws[:], in_=q_win[:])
        V_._custom_dve(RECIPROCAL_APPROX_NR, out=y0w[:], in0=q_win[:],
                       in1=y0ws[:], s0=2.0)
        tt(t_win[:], t_win[:], d_win[:], Alu.subtract)
        tt(t_win[:], t_win[:], y0w[:], Alu.mult)
        w8r = small.tile([R, 8], f32)
        V_.max(out=w8r[:], in_=t_win[:])
        wi8r = small.tile([R, 8], u32)
        V_.max_index(out=wi8r[:], in_max=w8r[:], in_values=t_win[:])
        wrf = small.tile([R, 1], f32)
        acast(wrf[:], wi8r[:, 0:1])
        rec = small.tile([R, 1], f32)
        ts(rec[:], fcf[:], float(G_CW), Alu.mult)
        tt(rec[:], rec[:], wrf[:], Alu.add)

        # ---- target window finish
        w8 = small.tile([128, 8], f32)
        V_.max(out=w8[:], in_=wt[:])
        wi8 = small.tile([128, 8], u32)
        V_.max_index(out=wi8[:], in_max=w8[:], in_values=wt[:])
        wif = small.tile([128, 1], f32)
        acast(wif[:], wi8[:, 0:1])
        halfam = small.tile([128, 1], f32)
        ts(halfam[:], cif[:], float(G_CW), Alu.mult)
        tt(halfam[:], halfam[:], wif[:], Alu.add)
        pk2 = small.tile([128, 2], f32)
        V_.tensor_copy(out=pk2[:, 0:1], in_=hm[:])
        V_.tensor_copy(out=pk2[:, 1:2], in_=halfam[:])
        comb = small.tile([ROWS, 4], f32)   # (lo_m, lo_am, hi_m, hi_am)
        G_.dma_start(comb[:], pk2[:])
        win_hi = small.tile([ROWS, 1], i32)
        tt(win_hi[:], comb[:, 2:3], comb[:, 0:1], Alu.is_gt)
        am_hi = small.tile([ROWS, 1], f32)
        ts(am_hi[:], comb[:, 3:4], float(G_HALF), Alu.add)
        am64 = small.tile([ROWS, 1], f32)
        V_.tensor_copy(out=am64[:], in_=comb[:, 1:2])
        V_.copy_predicated(am64[:], win_hi[:], am_hi[:])
        tgt_am = small.tile([R, S], f32)
        G_.dma_start(tgt_am[:], am64[:])

        # ---- greedy logic
        mism = small.tile([R, S], f32)
        tt(mism[:], tokf[:], tgt_am[:], Alu.not_equal)
        tt(mism[:], mism[:], validj[:], Alu.mult)
        mmrev = small.tile([R, S], f32)
        tt(mmrev[:], mism[:], rev8f[:], Alu.mult)
        mm_max = small.tile([R, 1], f32)
        V_.tensor_reduce(mm_max[:], mmrev[:], axis=AX.X, op=Alu.max)
        first_mm = small.tile([R, 1], f32)
        ts(first_mm[:], mm_max[:], -1.0, Alu.mult, float(S), Alu.add)
        copy_len = small.tile([R, 1], f32)
        ts(copy_len[:], first_mm[:], 1.0, Alu.add)
        tt(copy_len[:], copy_len[:], nperf[:], Alu.min)

        # ---- output assembly
        draft9 = small.tile([R, SP1], f32)
        V_.memset(draft9[:, S:SP1], 0.0)
        V_.tensor_copy(out=draft9[:, 0:S], in_=tokf[:])
        am9 = small.tile([R, SP1], f32)
        V_.memset(am9[:, S:SP1], 0.0)
        V_.tensor_copy(out=am9[:, 0:S], in_=tgt_am[:])
        validj9 = small.tile([R, SP1], f32)
        ts(validj9[:], j9f[:], nperf[:, 0:1], Alu.is_lt)
        jeqn = small.tile([R, SP1], f32)
        ts(jeqn[:], j9f[:], nperf[:, 0:1], Alu.is_equal)
        on = small.tile([R, SP1], f32)
        acast(on[:], oin_i)
        dmask = small.tile([R, SP1], i32)
        stt(dmask[:], j9f[:], first_rej[:, 0:1], Alu.is_lt, validj9[:], Alu.mult)
        V_.copy_predicated(on[:], dmask[:], draft9[:])
        rmask = small.tile([R, SP1], i32)
        stt(rmask[:], j9f[:], first_rej[:, 0:1], Alu.is_equal, validj9[:], Alu.mult)
        V_.copy_predicated(on[:], rmask[:], rec[:].to_broadcast([R, SP1]))
        bn = small.tile([R, 1], f32)
        tt(bn[:], first_rej[:], nperf[:], Alu.is_ge)
        bmn = small.tile([R, SP1], i32)
        tt(bmn[:], jeqn[:], bn[:].to_broadcast([R, SP1]), Alu.mult)
        V_.copy_predicated(on[:], bmn[:], bonf[:].to_broadcast([R, SP1]))
        og = small.tile([R, SP1], f32)
        acast(og[:], oin_i)
        clm = small.tile([R, SP1], i32)
        ts(clm[:], j9f[:], copy_len[:, 0:1], Alu.is_lt)
        V_.copy_predicated(og[:], clm[:], am9[:])
        bg = small.tile([R, 1], f32)
        tt(bg[:], first_mm[:], nperf[:], Alu.is_ge)
        bmg = small.tile([R, SP1], i32)
        tt(bmg[:], jeqn[:], bg[:].to_broadcast([R, SP1]), Alu.mult)
        V_.copy_predicated(og[:], bmg[:], bonf[:].to_broadcast([R, SP1]))
        isg9 = small.tile([R, SP1], i32)
        V_.tensor_copy(out=isg9[:], in_=isgi.to_broadcast([R, SP1]))
        V_.copy_predicated(on[:], isg9[:], og[:])
        outi = small.tile([R, SP1], i32)
        V_.tensor_copy(out=outi[:], in_=on[:])
        nc.sync.dma_start(out[:, :], outi[:])





def shard_inputs_gen(inputs, n_cores=8):
    cu = inputs["cu_num_draft_tokens"].astype(np.int64)
    B = cu.shape[0]
    N = inputs["draft_token_ids"].shape[0]
    n_per = np.diff(np.concatenate([[0], cu]))
    start = cu - n_per
    Rc = B // n_cores
    gidx = np.clip(start[:, None] + np.arange(S)[None, :], 0, N - 1)
    in_maps = []
    for c in range(n_cores):
        rs = slice(c * Rc, (c + 1) * Rc)
        row0 = int(start[c * Rc])
        idx = np.arange(row0, row0 + ROWS)
        if idx[-1] < N:
            tgt_c = inputs["target_probs"][row0:row0 + ROWS]
            drf_c = inputs["draft_probs"][row0:row0 + ROWS]
        else:
            idxc = np.clip(idx, 0, N - 1)
            tgt_c = inputs["target_probs"][idxc]
            drf_c = inputs["draft_probs"][idxc]
        g = gidx[rs]
        pkc = np.zeros((R, G_PKW), np.int32)
        pkc[:, G_C_TOK:G_C_TOK + 8] = inputs["draft_token_ids"][g].astype(np.int32)
        pkc[:, G_C_U:G_C_U + 8] = (
            inputs["uniform_probs"][g].astype(np.float32).view(np.int32))
        pkc[:, G_C_NPER] = n_per[rs].astype(np.int32)
        pkc[:, G_C_LST] = (start[rs] - row0).astype(np.int32)
        pkc[:, G_C_ISG] = inputs["is_greedy"][rs].astype(np.int32)
        pkc[:, G_C_BON] = inputs["bonus_token_ids"][rs].astype(np.int32)
        pkc[:, G_C_OIN:G_C_OIN + SP1] = inputs["output_token_ids"][rs].astype(np.int32)
        in_maps.append(dict(
            tgt=np.ascontiguousarray(tgt_c, dtype=np.float32),
            drf=np.ascontiguousarray(drf_c, dtype=np.float32),
            q=np.ascontiguousarray(inputs["q"][rs], dtype=np.float32),
            pk=pkc,
        ))
    return in_maps


def assemble_outputs_gen(results):
    return np.concatenate([r["out"] for r in results], axis=0).astype(np.int32)


# ---------------- dispatch ----------------

_CACHE = {}


def _get_nc(kind):
    if kind not in _CACHE:
        if kind == "fast":
            _CACHE[kind] = build_kernel_fast(n_devices=8)
        else:
            _CACHE[kind] = build_kernel_gen(n_devices=8)
    return _CACHE[kind]


def _kernel_numpy(output_token_ids, cu_num_draft_tokens, draft_token_ids,
                  draft_probs, target_probs, bonus_token_ids, uniform_probs,
                  q, is_greedy):
    """Shape-agnostic reference fallback (host compute; only used for inputs
    the compiled device programs cannot fit)."""
    out = np.array(output_token_ids, dtype=np.int32).copy()
    Bb, Sp1 = out.shape
    Sl = Sp1 - 1
    Nt = draft_token_ids.shape[0]
    cu = np.asarray(cu_num_draft_tokens, dtype=np.int64)
    n_per = np.diff(np.concatenate([[0], cu]))
    start_ = cu - n_per
    tam = target_probs.argmax(axis=-1).astype(np.int32)
    prob = np.maximum(target_probs - draft_probs, 0.0)
    req_id = np.searchsorted(cu, np.arange(Nt), side="right")
    rec = (prob / q[req_id]).argmax(axis=1).astype(np.int32)
    for r in range(Bb):
        npr = int(n_per[r]); st = int(start_[r])
        if is_greedy[r]:
            k = npr
            for j in range(npr):
                g = min(st + j, Nt - 1)
                if draft_token_ids[g] != tam[g]:
                    k = j
                    break
            for j in range(min(k + 1, npr)):
                out[r, j] = tam[min(st + j, Nt - 1)]
            if k >= npr and npr < Sp1:
                out[r, npr] = bonus_token_ids[r]
        else:
            fr = Sl
            for j in range(npr):
                g = min(st + j, Nt - 1)
                dp = draft_probs[g, draft_token_ids[g]]
                tp = target_probs[g, draft_token_ids[g]]
                ok = dp > 0 and (tp / dp) >= uniform_probs[g]
                if not ok:
                    fr = j
                    break
            for j in range(npr):
                g = min(st + j, Nt - 1)
                if j < fr:
                    out[r, j] = draft_token_ids[g]
                elif j == fr:
                    out[r, j] = rec[g]
                else:
                    break
            if fr >= npr and npr < Sp1:
                out[r, npr] = bonus_token_ids[r]
    return out


def _shapes_ok(inputs):
    try:
        return (inputs["output_token_ids"].shape == (64, 9)
                and inputs["cu_num_draft_tokens"].shape == (64,)
                and inputs["draft_token_ids"].shape == (512,)
                and inputs["draft_probs"].shape == (512, 32000)
                and inputs["target_probs"].shape == (512, 32000)
                and inputs["bonus_token_ids"].shape == (64,)
                and inputs["uniform_probs"].shape == (512,)
                and inputs["q"].shape == (64, 32000)
                and inputs["is_greedy"].shape == (64,))
    except Exception:
        return False


def kernel(**inputs):
    inputs = {k: np.asarray(v) for k, v in inputs.items()}
    if not _shapes_ok(inputs):
        return _kernel_numpy(**inputs)
    cu = inputs["cu_num_draft_tokens"].astype(np.int64)
    n_per = np.diff(np.concatenate([[0], cu]))
    uniform = bool((n_per == S).all())
    perm = plan_permutation(inputs) if uniform else None
    if perm is not None:
        nc = _get_nc("fast")
        in_maps = shard_inputs_fast(inputs, perm)
        res = bass_utils.run_bass_kernel_spmd(nc, in_maps,
                                              core_ids=list(range(8)))
        return assemble_outputs_fast(res.results, perm)
    if bool((n_per >= 0).all()) and bool((n_per <= S).all()):
        nc = _get_nc("gen")
        in_maps = shard_inputs_gen(inputs)
        res = bass_utils.run_bass_kernel_spmd(nc, in_maps,
                                              core_ids=list(range(8)))
        return assemble_outputs_gen(res.results)
    return _kernel_numpy(**inputs)



# revision 7
# speedup vs baseline: 1.1672x; 1.1672x over previous
"""AscendRejectionSampler — Trainium2 Bass kernel (8-core SPMD), v2.

kernel(**inputs) takes the full unsharded inputs and returns the full
[64, 9] int32 output.

Sharding: data-parallel over requests, 8 requests per core, balanced as
4 non-greedy (slots 0-3) + 4 greedy (slots 4-7).  Device program:

- tgt+drf uploaded as ONE [128, 32000] tensor (rows 0-63 target, 64-127
  draft) so dp/tp and the t/d segment reads are single multi-offset
  indirect gathers;
- target-probs big load split into 4 chunks with the chunk-max reduces
  interleaved on DVE;
- every cross-partition step uses DVE 32x32 block-transposes and tiny
  one-hot fp32 PE matmuls instead of DMA shuffles;
- approx reciprocal without NR refinement;
- all constants / masks / f32 scalars packed host-side into one int32
  tensor pkall [128, *].
"""
import sys
sys.path.insert(0, '/opt/trn_rl_repo')
import numpy as np
import concourse.bass as bass
import concourse.bacc as bacc
import concourse.tile as tile
from concourse import mybir
from concourse import bass_utils

f32 = mybir.dt.float32
i32 = mybir.dt.int32
u32 = mybir.dt.uint32
Alu = mybir.AluOpType
AX = mybir.AxisListType

R = 8
S = 8
SP1 = 9
V = 32000
ROWS = 64
NG = 4            # slots 0-3 non-greedy, 4-7 greedy
GROWS = 32        # greedy token rows (td rows 32..64)
QW = 8000         # quarter width
CW = 125          # chunk width
NCHQ = QW // CW   # 64 chunks per quarter
SEGW = 1000       # seg width (32 segs per row)
TLC = 2000        # target-load chunk width (per partition), x4

# pkall [128, PKALLW].  Three regions:
# cols 0..C64: per-token-partition data (rows 0-63)
P64_OFF = 0       # i32 [2]: tp offset p*V+tok, dp offset (64+p)*V+tok
P64_U = 2
P64_VREV = 3      # (j<nper)*(8-j)
P64_MSEL = 4      # [4]: p<32: (s == p//8)
P64_BASE = 8      # p<32: (p//8)*256 + 256
C64 = 9
# cols C64..C8: per-slot data (rows 0-7)
P8_TOK9 = C64         # [9] draft tokens as f32 (col 8 = 0)
P8_NPER = C64 + 9
P8_BON = C64 + 10
P8_OIN = C64 + 11     # [9]
P8_J9 = C64 + 20      # [9]
P8_V9 = C64 + 29      # [9] j < nper
P8_JEQ = C64 + 38     # [9] j == nper
P8_VREV8 = C64 + 47   # [8] (j<nper)*(8-j)
P8_ISG9 = C64 + 55    # [9] i32 is_greedy mask
P8_M2D = C64 + 64     # [4] (c == p)
C8 = C64 + 68
# cols C8..: per-128-partition constants
P128_W0 = C8          # i32: (32 + p//4)*256 + (p%4)*64
P128_QOFF = C8 + 1    # (p%4)*8000
P128_R8 = C8 + 2      # [8]: (r == (p%32)//4)
P128_REVQ = C8 + 10   # 4 - p%4
P128_LTGT = C8 + 11   # [8]: (m == 4 + p//32)
P128_SEGB = C8 + 19   # (p%32)*1000
P128_REVS = C8 + 20   # 32 - p%32
P128_LSEG = C8 + 21   # [8]: (m == p//32)
P128_SEGID = C8 + 29  # p%32
P128_LFR = C8 + 30    # [128], rows p<32: -32*(p == 8*(m//32))
P128_SEGID2 = C8 + 158   # (p//32)*256 + 256 + p%32   (ng soffs affine)
P128_SEGID2B = C8 + 159  # same + 2048                (draft-half row)
P128_R32 = C8 + 160      # [32]: (j//4 == (p%32)//4)
P128_QID = C8 + 192      # p%4
PKALLW = C8 + 193


def build_kernel_fast(n_devices=8):
    nc = bacc.Bacc("TRN2", target_bir_lowering=False, debug=False,
                   enable_asserts=False, num_devices=n_devices)
    td = nc.dram_tensor("td", [128, V], f32, kind="ExternalInput").ap()
    q4 = nc.dram_tensor("q4", [NG, V], f32, kind="ExternalInput").ap()
    pko = nc.dram_tensor("pko", [ROWS, 2], i32, kind="ExternalInput").ap()
    pkall = nc.dram_tensor("pkall", [128, PKALLW], i32,
                           kind="ExternalInput").ap()
    out = nc.dram_tensor("out", [R, SP1], i32, kind="ExternalOutput").ap()
    with tile.TileContext(nc) as tc:
        _body(tc, nc, td, q4, pko, pkall, out)
    nc.compile()
    return nc


def _body(tc, nc, td, q4, pko, pkall, out):
    import contextlib
    ctx = contextlib.ExitStack()
    with ctx:
        small = ctx.enter_context(tc.tile_pool(name="small", bufs=1))
        big = ctx.enter_context(tc.tile_pool(name="big", bufs=1))
        psum = ctx.enter_context(tc.tile_pool(name="psum", bufs=1,
                                              space="PSUM"))

        V_ = nc.vector
        G_ = nc.gpsimd
        A_ = nc.scalar

        def ts(out_, in_, s1, op0, s2=None, op1=None):
            if op1 is None:
                return V_.tensor_scalar(out_, in_, s1, None, op0=op0)
            return V_.tensor_scalar(out_, in_, s1, s2, op0=op0, op1=op1)

        def tt(out_, a, b, op):
            return V_.tensor_tensor(out_, a, b, op=op)

        def stt(out_, in0, scalar, op0, in1, op1):
            return V_.scalar_tensor_tensor(out_, in0, scalar, in1, op0=op0,
                                           op1=op1)

        # ---------------- sync (HWDGE) loads, issue order = queue order
        pkot = small.tile([ROWS, 2], i32)
        nc.sync.dma_start(pkot[:], pko[:, :])
        pkt = small.tile([128, PKALLW], i32)
        nc.sync.dma_start(pkt[:], pkall[:, :])
        tgtv = td[GROWS:ROWS, :].rearrange("a (h c) -> (a h) c", c=QW)
        tl = []
        for k in range(4):
            t = big.tile([128, TLC], f32, name=f"tl{k}")
            nc.sync.dma_start(t[:], tgtv[:, k * TLC:(k + 1) * TLC])
            tl.append(t)

        # ---------------- views into the packed tile
        p8 = pkt[0:R, :]
        tokf9 = p8[:, P8_TOK9:P8_TOK9 + SP1].bitcast(f32)
        tokf8 = p8[:, P8_TOK9:P8_TOK9 + S].bitcast(f32)
        nperf = p8[:, P8_NPER:P8_NPER + 1].bitcast(f32)
        bonf = p8[:, P8_BON:P8_BON + 1].bitcast(f32)
        oinf9 = p8[:, P8_OIN:P8_OIN + SP1].bitcast(f32)
        j9f = p8[:, P8_J9:P8_J9 + SP1].bitcast(f32)
        validj9 = p8[:, P8_V9:P8_V9 + SP1].bitcast(f32)
        jeqn = p8[:, P8_JEQ:P8_JEQ + SP1].bitcast(f32)
        vrev8 = p8[:, P8_VREV8:P8_VREV8 + S].bitcast(f32)
        isg9 = p8[:, P8_ISG9:P8_ISG9 + SP1]
        m2diag = p8[:, P8_M2D:P8_M2D + NG].bitcast(f32)

        p64 = pkt[0:ROWS, :]
        offs2 = p64[:, P64_OFF:P64_OFF + 2]
        u64 = p64[:, P64_U:P64_U + 1].bitcast(f32)
        vrev64 = p64[:, P64_VREV:P64_VREV + 1].bitcast(f32)
        msel = p64[:, P64_MSEL:P64_MSEL + NG].bitcast(f32)
        basecol = p64[:, P64_BASE:P64_BASE + 1].bitcast(f32)

        woffs0 = pkt[:, P128_W0:P128_W0 + 1]
        qoffs = pkt[:, P128_QOFF:P128_QOFF + 1].bitcast(f32)
        r8m = pkt[:, P128_R8:P128_R8 + 8].bitcast(f32)
        revq = pkt[:, P128_REVQ:P128_REVQ + 1].bitcast(f32)
        lhsTtgt = pkt[:, P128_LTGT:P128_LTGT + 8].bitcast(f32)
        segbase = pkt[:, P128_SEGB:P128_SEGB + 1].bitcast(f32)
        revseg = pkt[:, P128_REVS:P128_REVS + 1].bitcast(f32)
        lhsTseg = pkt[:, P128_LSEG:P128_LSEG + 8].bitcast(f32)
        segid = pkt[:, P128_SEGID:P128_SEGID + 1].bitcast(f32)
        lhsTfr = pkt[0:32, P128_LFR:P128_LFR + 128].bitcast(f32)
        segid2 = pkt[:, P128_SEGID2:P128_SEGID2 + 1].bitcast(f32)
        segid2b = pkt[:, P128_SEGID2B:P128_SEGID2B + 1].bitcast(f32)
        r32m = pkt[:, P128_R32:P128_R32 + 32].bitcast(f32)
        qidf = pkt[:, P128_QID:P128_QID + 1].bitcast(f32)

        # ---------------- gpsimd: tp/dp gathers, then q4 on the idle queue
        tdflat = td.rearrange("a b -> (a b)").unsqueeze(-1)
        tp64 = small.tile([ROWS, 1], f32)
        G_.indirect_dma_start(
            out=tp64[:], out_offset=None, in_=tdflat,
            in_offset=bass.IndirectOffsetOnAxis(ap=pkot[:, 0:1], axis=0))
        dp64 = small.tile([ROWS, 1], f32)
        G_.indirect_dma_start(
            out=dp64[:], out_offset=None, in_=tdflat,
            in_offset=bass.IndirectOffsetOnAxis(ap=pkot[:, 1:2], axis=0))
        q4t = big.tile([128, SEGW], f32)
        G_.dma_start(q4t[:], q4.rearrange("a (h c) -> (a h) c", c=SEGW))

        # ---------------- early memsets (DVE)
        am9 = small.tile([R, SP1], f32)
        V_.memset(am9[:, S:SP1], 0.0)
        inmax8 = small.tile([128, 8], f32)
        V_.memset(inmax8[:, 1:8], -3.0e38)
        rv8 = small.tile([128, 8], f32)
        V_.memset(rv8[:, 4:8], -1.0)

        # ---------------- DVE: first chunk reduce fills the dp/tp wait
        cmax = small.tile([128, NCHQ], f32)
        NCC = TLC // CW  # 16 chunks per tl chunk
        V_.tensor_reduce(cmax[:, 0:NCC],
                         tl[0][:].rearrange("p (b c) -> p b c", c=CW),
                         axis=AX.X, op=Alu.max)

        # ---------------- DVE: accept logic in [64,1] layout
        # accept <=> dp*u <= tp (dp > 0 always for softmax inputs)
        c2 = small.tile([ROWS, 1], f32)
        stt(c2[:], dp64[:], u64, Alu.mult, tp64[:], Alu.is_le)
        rejrev = small.tile([ROWS, 1], f32)
        stt(rejrev[:], c2[:], 0.0, Alu.is_equal, vrev64, Alu.mult)

        # ---------------- first_rej per slot via block transpose
        b64 = small.tile([ROWS, 32], f32)
        V_.tensor_copy(out=b64[:], in_=rejrev[:].to_broadcast([ROWS, 32]))
        t64 = small.tile([ROWS, 32], f32)
        V_.transpose(out=t64[:], in_=b64[:])
        mrev_all = small.tile([ROWS, NG], f32)
        V_.tensor_reduce(mrev_all[:],
                         t64[:].rearrange("p (a b) -> p a b", b=8),
                         axis=AX.X, op=Alu.max)
        scr64 = small.tile([ROWS, NG], f32)
        tt(scr64[:], mrev_all[:], msel, Alu.mult)
        mrevsel = small.tile([ROWS, 1], f32)
        V_.tensor_reduce(mrevsel[:], scr64[:], axis=AX.X, op=Alu.max)

        # -32*mrev(slot) distributed to all 128 partitions via one-hot
        # matmul (scale folded into lhsT); affine finish + i32 cast on ACT
        base128 = psum.tile([128, 1], f32)
        nc.tensor.matmul(base128[:], lhsTfr, mrevsel[0:32, 0:1])
        soffs2 = small.tile([128, 2], i32)
        A_.activation(soffs2[:, 0:1], base128[:],
                      mybir.ActivationFunctionType.Identity, bias=segid2)
        A_.activation(soffs2[:, 1:2], base128[:],
                      mybir.ActivationFunctionType.Identity, bias=segid2b)

        # ---------------- gpsimd: t/d seg gathers + diff
        tdv = td.rearrange("a (b c) -> (a b) c", c=SEGW)
        t_seg = big.tile([128, SEGW], f32)
        G_.indirect_dma_start(
            out=t_seg[:], out_offset=None, in_=tdv,
            in_offset=bass.IndirectOffsetOnAxis(ap=soffs2[:, 0:1], axis=0))
        d_seg = big.tile([128, SEGW], f32)
        G_.indirect_dma_start(
            out=d_seg[:], out_offset=None, in_=tdv,
            in_offset=bass.IndirectOffsetOnAxis(ap=soffs2[:, 1:2], axis=0))


        # ---------------- DVE: remaining chunk reduces + reciprocal
        V_.tensor_reduce(cmax[:, NCC:2 * NCC],
                         tl[1][:].rearrange("p (b c) -> p b c", c=CW),
                         axis=AX.X, op=Alu.max)
        rq = big.tile([128, SEGW], f32)
        V_.reciprocal_approx_fast(out=rq[:], in_=q4t[:])
        for k in (2, 3):
            V_.tensor_reduce(cmax[:, k * NCC:(k + 1) * NCC],
                             tl[k][:].rearrange("p (b c) -> p b c", c=CW),
                             axis=AX.X, op=Alu.max)

        # ---------------- DVE: ratio scan
        diff = big.tile([128, SEGW], f32)
        tt(diff[:], t_seg[:], d_seg[:], Alu.subtract)
        ratio = big.tile([128, SEGW], f32)
        tt(ratio[:], diff[:], rq[:], Alu.mult)
        V_.tensor_reduce(inmax8[:, 0:1], ratio[:], axis=AX.X, op=Alu.max)
        is8 = small.tile([128, 8], u32)
        V_.max_index(out=is8[:], in_max=inmax8[:], in_values=ratio[:])
        i_f = small.tile([128, 1], f32)
        A_.copy(out=i_f[:], in_=is8[:, 0:1])

        # ---------------- target argmax: chunk pick + window gather
        c8 = small.tile([128, 8], f32)
        V_.max(out=c8[:], in_=cmax[:])
        ci8 = small.tile([128, 8], u32)
        V_.max_index(out=ci8[:], in_max=c8[:], in_values=cmax[:])
        ci_i = small.tile([128, 1], i32)
        A_.copy(out=ci_i[:], in_=ci8[:, 0:1])
        cif = small.tile([128, 1], f32)
        A_.copy(out=cif[:], in_=ci_i[:])
        woffs = small.tile([128, 1], i32)
        tt(woffs[:], woffs0, ci_i[:], Alu.add)
        wt = small.tile([128, CW], f32)
        G_.indirect_dma_start(
            out=wt[:], out_offset=None,
            in_=td.rearrange("a (b c) -> (a b) c", c=CW),
            in_offset=bass.IndirectOffsetOnAxis(ap=woffs[:], axis=0))

        # ---------------- seg argmax finish (band == slot)
        # transpose gives every partition its band's 32 seg-maxes; the
        # band argmax (first occurrence) is the winning seg id
        mseg = inmax8[:, 0:1]
        bs = small.tile([128, 32], f32)
        V_.tensor_copy(out=bs[:], in_=mseg.to_broadcast([128, 32]))
        ts_t = small.tile([128, 32], f32)
        V_.transpose(out=ts_t[:], in_=bs[:])
        sx8 = small.tile([128, 8], f32)
        V_.max(out=sx8[:], in_=ts_t[:])
        si8 = small.tile([128, 8], u32)
        V_.max_index(out=si8[:], in_max=sx8[:], in_values=ts_t[:])
        s_f = small.tile([128, 1], f32)
        A_.copy(out=s_f[:], in_=si8[:, 0:1])
        iswin = small.tile([128, 1], f32)
        tt(iswin[:], segid, s_f[:], Alu.is_equal)
        recval = small.tile([128, 1], f32)
        ts(recval[:], i_f[:], segbase, Alu.add)
        rec_p = small.tile([128, 1], f32)
        tt(rec_p[:], recval[:], iswin[:], Alu.mult)
        rec8p = psum.tile([R, 1], f32)
        nc.tensor.matmul(rec8p[:], lhsTseg, rec_p[:])
        rec_sb = small.tile([R, 1], f32)
        A_.copy(out=rec_sb[:], in_=rec8p[:])

        # ---------------- cross-quarter winner while the window gather flies
        # transpose + own-row mask + strided reduce -> the row's 4 quarter
        # maxes per partition; argmax (first occurrence) = winning quarter
        m128 = c8[:, 0:1]
        bt = small.tile([128, 32], f32)
        V_.tensor_copy(out=bt[:], in_=m128.to_broadcast([128, 32]))
        tt_t = small.tile([128, 32], f32)
        V_.transpose(out=tt_t[:], in_=bt[:])
        prod = small.tile([128, 32], f32)
        tt(prod[:], tt_t[:], r32m, Alu.mult)
        V_.tensor_reduce(rv8[:, 0:4],
                         prod[:].rearrange("p (a b) -> p b a", b=4),
                         axis=AX.X, op=Alu.max)
        qx8 = small.tile([128, 8], f32)
        V_.max(out=qx8[:], in_=rv8[:])
        qi8 = small.tile([128, 8], u32)
        V_.max_index(out=qi8[:], in_max=qx8[:], in_values=rv8[:])
        q_f = small.tile([128, 1], f32)
        A_.copy(out=q_f[:], in_=qi8[:, 0:1])
        iswint = small.tile([128, 1], f32)
        tt(iswint[:], qidf, q_f[:], Alu.is_equal)

        # window finish
        w8 = small.tile([128, 8], f32)
        V_.max(out=w8[:], in_=wt[:])
        wi8 = small.tile([128, 8], u32)
        V_.max_index(out=wi8[:], in_max=w8[:], in_values=wt[:])
        wif = small.tile([128, 1], f32)
        A_.copy(out=wif[:], in_=wi8[:, 0:1])
        qam = small.tile([128, 1], f32)
        ts(qam[:], cif[:], float(CW), Alu.mult, qoffs, Alu.add)
        tt(qam[:], qam[:], wif[:], Alu.add)
        amw = small.tile([128, 1], f32)
        tt(amw[:], qam[:], iswint[:], Alu.mult)
        rhs_tgt = small.tile([128, 8], f32)
        tt(rhs_tgt[:], amw[:].to_broadcast([128, 8]), r8m, Alu.mult)
        tgtamp = psum.tile([R, S], f32)
        nc.tensor.matmul(tgtamp[:], lhsTtgt, rhs_tgt[:])
        A_.copy(out=am9[:, 0:S], in_=tgtamp[:])

        # ---------------- greedy logic [8,8]
        mism = small.tile([R, S], f32)
        tt(mism[:], tokf8, tgtamp[:], Alu.not_equal)
        mmrev = small.tile([R, S], f32)
        tt(mmrev[:], mism[:], vrev8, Alu.mult)
        mm_max = small.tile([R, 1], f32)
        V_.tensor_reduce(mm_max[:], mmrev[:], axis=AX.X, op=Alu.max)
        first_mm = small.tile([R, 1], f32)
        ts(first_mm[:], mm_max[:], -1.0, Alu.mult, float(S), Alu.add)
        copy_len = small.tile([R, 1], f32)
        ts(copy_len[:], first_mm[:], 1.0, Alu.add)
        tt(copy_len[:], copy_len[:], nperf, Alu.min)

        # first_rej per slot for the [8,*] assembly
        scr8 = small.tile([R, NG], f32)
        tt(scr8[:], mrev_all[0:R, :], m2diag, Alu.mult)
        mrevdiag = small.tile([R, 1], f32)
        V_.tensor_reduce(mrevdiag[:], scr8[:], axis=AX.X, op=Alu.max)
        fr8 = small.tile([R, 1], f32)
        ts(fr8[:], mrevdiag[:], -1.0, Alu.mult, float(S), Alu.add)

        # ---------------- output assembly
        on = small.tile([R, SP1], f32)
        A_.copy(out=on[:], in_=oinf9)
        og = small.tile([R, SP1], f32)
        A_.copy(out=og[:], in_=oinf9)
        dmask = small.tile([R, SP1], i32)
        stt(dmask[:], j9f, fr8[:, 0:1], Alu.is_lt, validj9, Alu.mult)
        V_.copy_predicated(on[:], dmask[:], tokf9)
        rmask = small.tile([R, SP1], i32)
        stt(rmask[:], j9f, fr8[:, 0:1], Alu.is_equal, validj9, Alu.mult)
        V_.copy_predicated(on[:], rmask[:], rec_sb[:].to_broadcast([R, SP1]))
        bmn = small.tile([R, SP1], i32)
        stt(bmn[:], fr8[:].to_broadcast([R, SP1]), nperf[:, 0:1], Alu.is_ge,
            jeqn, Alu.mult)
        V_.copy_predicated(on[:], bmn[:], bonf.to_broadcast([R, SP1]))
        clm = small.tile([R, SP1], i32)
        ts(clm[:], j9f, copy_len[:, 0:1], Alu.is_lt)
        V_.copy_predicated(og[:], clm[:], am9[:])
        bmg = small.tile([R, SP1], i32)
        stt(bmg[:], first_mm[:].to_broadcast([R, SP1]), nperf[:, 0:1],
            Alu.is_ge, jeqn, Alu.mult)
        V_.copy_predicated(og[:], bmg[:], bonf.to_broadcast([R, SP1]))
        V_.copy_predicated(on[:], isg9, og[:])
        outi = small.tile([R, SP1], i32)
        V_.tensor_copy(out=outi[:], in_=on[:])
        nc.sync.dma_start(out[:, :], outi[:])


# ---------------- host-side packing ----------------

def plan_permutation(inputs, n_cores=8):
    """Slot assignment: 4 non-greedy (slots 0-3) + 4 greedy (slots 4-7)."""
    isg = np.asarray(inputs["is_greedy"]).astype(bool)
    B = isg.shape[0]
    if B != ROWS or B // n_cores != R:
        return None
    g = np.where(isg)[0]
    n = np.where(~isg)[0]
    if len(g) != B // 2:
        return None
    perm = np.empty(B, np.int64)
    for c in range(n_cores):
        perm[c * R:c * R + NG] = n[c * NG:(c + 1) * NG]
        perm[c * R + NG:(c + 1) * R] = g[c * NG:(c + 1) * NG]
    return perm


def _f32bits(x):
    return np.asarray(x, dtype=np.float32).view(np.int32)


def _pk_static():
    """The per-128-partition constant columns (identical for all cores)."""
    p128 = np.arange(128)
    m8 = np.arange(8)[None, :]
    pk = np.zeros((128, PKALLW), np.int32)
    pk[:, P128_W0] = (GROWS + p128 // 4) * (V // CW) + (p128 % 4) * NCHQ
    pk[:, P128_QOFF] = _f32bits((p128 % 4) * QW)
    pk[:, P128_R8:P128_R8 + 8] = _f32bits((m8 == ((p128 % 32) // 4)[:, None]))
    pk[:, P128_REVQ] = _f32bits(4 - p128 % 4)
    pk[:, P128_LTGT:P128_LTGT + 8] = _f32bits((m8 == (4 + p128 // 32)[:, None]))
    pk[:, P128_SEGB] = _f32bits((p128 % 32) * SEGW)
    pk[:, P128_REVS] = _f32bits(32 - p128 % 32)
    pk[:, P128_LSEG:P128_LSEG + 8] = _f32bits((m8 == (p128 // 32)[:, None]))
    pk[:, P128_SEGID] = _f32bits(p128 % 32)
    kk = np.arange(32)[:, None]
    mm = np.arange(128)[None, :]
    pk[0:32, P128_LFR:P128_LFR + 128] = _f32bits(
        -32.0 * (kk == 8 * (mm // 32)))
    pk[:, P128_SEGID2] = _f32bits((p128 // 32) * 256 + 256 + p128 % 32)
    pk[:, P128_SEGID2B] = _f32bits((p128 // 32) * 256 + 256 + p128 % 32
                                   + 2048)
    j32 = np.arange(32)[None, :]
    pk[:, P128_R32:P128_R32 + 32] = _f32bits(
        (j32 // 4) == ((p128 % 32) // 4)[:, None])
    pk[:, P128_QID] = _f32bits(p128 % 4)
    # [64,*] static region
    mselm = np.zeros((ROWS, NG), np.float32)
    mselm[0:32] = (np.arange(NG)[None, :] == (np.arange(32) // 8)[:, None])
    pk[0:ROWS, P64_MSEL:P64_MSEL + NG] = _f32bits(mselm)
    basec = np.zeros(ROWS, np.float32)
    basec[0:32] = (np.arange(32) // 8) * 256 + 256
    pk[0:ROWS, P64_BASE] = _f32bits(basec)
    # [8,*] static region
    j9 = np.arange(SP1)[None, :]
    pk[0:R, P8_J9:P8_J9 + SP1] = _f32bits(np.broadcast_to(j9, (R, SP1)))
    pk[0:R, P8_M2D:P8_M2D + NG] = _f32bits(
        np.arange(NG)[None, :] == np.arange(R)[:, None])
    return pk


_PK_STATIC = None


def shard_inputs_fast(inputs, perm, n_cores=8):
    global _PK_STATIC
    if _PK_STATIC is None:
        _PK_STATIC = _pk_static()
    cu = inputs["cu_num_draft_tokens"].astype(np.int64)
    N = inputs["draft_token_ids"].shape[0]
    n_per = np.diff(np.concatenate([[0], cu]))
    start = cu - n_per
    gidx = np.clip(start[:, None] + np.arange(S)[None, :], 0, N - 1)  # [B,S]
    jj = np.arange(S)[None, :]
    j9 = np.arange(SP1)[None, :]
    p64 = np.arange(ROWS)

    in_maps = []
    for c in range(n_cores):
        reqs = perm[c * R:(c + 1) * R]
        g = gidx[reqs]                     # [8, 8] global token-row ids
        rows = g.reshape(-1)
        npr = n_per[reqs][:, None]         # [8, 1]
        tokc = inputs["draft_token_ids"][g].astype(np.int64)   # [8, 8]

        pk = _PK_STATIC.copy()
        tok9 = np.zeros((R, SP1), np.float32)
        tok9[:, 0:S] = tokc
        pk[0:R, P8_TOK9:P8_TOK9 + SP1] = _f32bits(tok9)
        pk[0:R, P8_NPER] = _f32bits(n_per[reqs])
        pk[0:R, P8_BON] = _f32bits(inputs["bonus_token_ids"][reqs])
        pk[0:R, P8_OIN:P8_OIN + SP1] = _f32bits(
            inputs["output_token_ids"][reqs])
        pk[0:R, P8_V9:P8_V9 + SP1] = _f32bits(j9 < npr)
        pk[0:R, P8_JEQ:P8_JEQ + SP1] = _f32bits(j9 == npr)
        pk[0:R, P8_VREV8:P8_VREV8 + S] = _f32bits((jj < npr) * (S - jj))
        pk[0:R, P8_ISG9:P8_ISG9 + SP1] = (
            inputs["is_greedy"][reqs].astype(np.int32)[:, None]
            * np.ones((1, SP1), np.int32))

        flat_tok = tokc.reshape(-1)
        pkoc = np.empty((ROWS, 2), np.int32)
        pkoc[:, 0] = (p64 * V + flat_tok).astype(np.int32)
        pkoc[:, 1] = ((p64 + ROWS) * V + flat_tok).astype(np.int32)
        pk[0:ROWS, P64_U] = _f32bits(inputs["uniform_probs"][g].reshape(-1))
        vrev = ((jj < npr) * (S - jj)).astype(np.float32)
        pk[0:ROWS, P64_VREV] = _f32bits(vrev.reshape(-1))

        tdc = np.empty((128, V), np.float32)
        tdc[0:ROWS] = inputs["target_probs"][rows]
        tdc[ROWS:128] = inputs["draft_probs"][rows]
        in_maps.append(dict(
            td=tdc,
            q4=np.ascontiguousarray(inputs["q"][reqs[0:NG]],
                                    dtype=np.float32),
            pko=pkoc,
            pkall=pk,
        ))
    return in_maps


def assemble_outputs_fast(results, perm):
    raw = np.concatenate([r["out"] for r in results], axis=0).astype(np.int32)
    out = np.empty_like(raw)
    out[perm] = raw
    return out


from concourse.dve_ops import RECIPROCAL_APPROX_NR
from concourse.tile import add_dep_helper

G_TW = 8000
G_VT = 2
G_CW = 125
G_NCH = 16000 // G_CW
G_NWIN = V // G_CW
G_SEGW = 2000
G_HALF = 16000
G_C_TOK, G_C_U, G_C_NPER, G_C_LST, G_C_ISG, G_C_BON, G_C_OIN = 0, 8, 16, 17, 18, 19, 20
G_PKW = 29



def build_kernel_gen(n_devices=8):
    nc = bacc.Bacc("TRN2", target_bir_lowering=False, debug=False,
                   enable_asserts=True, num_devices=n_devices)
    tgt = nc.dram_tensor("tgt", [ROWS, V], f32, kind="ExternalInput").ap()
    drf = nc.dram_tensor("drf", [ROWS, V], f32, kind="ExternalInput").ap()
    q = nc.dram_tensor("q", [R, V], f32, kind="ExternalInput").ap()
    pk = nc.dram_tensor("pk", [R, G_PKW], i32, kind="ExternalInput").ap()
    out = nc.dram_tensor("out", [R, SP1], i32, kind="ExternalOutput").ap()
    import os as _os
    dbg = None
    if _os.environ.get("RSK_DBG"):
        dbg = nc.dram_tensor("dbg", [R, 4 * S], f32, kind="ExternalOutput").ap()
    with tile.TileContext(nc) as tc:
        _body_gen(tc, nc, tgt, drf, q, pk, out, dbg)
    nc.compile()
    return nc


def _body_gen(tc, nc, tgt, drf, q, pk, out, dbg=None):
    import contextlib
    ctx = contextlib.ExitStack()
    with ctx:
        small = ctx.enter_context(tc.tile_pool(name="small", bufs=1))
        stream = ctx.enter_context(tc.tile_pool(name="stream", bufs=1))
        segp = ctx.enter_context(tc.tile_pool(name="segp", bufs=1))

        V_ = nc.vector
        G_ = nc.gpsimd

        def ts(out_, in_, s1, op0, s2=None, op1=None):
            if op1 is None:
                V_.tensor_scalar(out_, in_, s1, None, op0=op0)
            else:
                V_.tensor_scalar(out_, in_, s1, s2, op0=op0, op1=op1)

        def tt(out_, a, b, op):
            V_.tensor_tensor(out_, a, b, op=op)

        def stt(out_, in0, scalar, op0, in1, op1):
            V_.scalar_tensor_tensor(out_, in0, scalar, in1, op0=op0, op1=op1)

        def acast(dst, src):
            nc.scalar.copy(out=dst, in_=src)

        # ---- sync ring: pk, q, tileA (the rest of sync is the final store)
        pkt = small.tile([R, G_PKW], i32)
        nc.sync.dma_start(pkt[:], pk[:, :])
        q_seg = segp.tile([128, G_SEGW], f32)
        nc.sync.dma_start(q_seg[:], q[:, :])
        tgt_v = tgt.rearrange("a (h c) -> (a h) c", c=G_HALF)  # [128, 16000]
        tlA = stream.tile([128, G_TW], f32, tag="tlA", name="tlA")
        nc.sync.dma_start(tlA[:], tgt_v[:, 0:G_TW])

        tok88i = pkt[:, G_C_TOK:G_C_TOK + 8]
        u88 = pkt[:, G_C_U:G_C_U + 8].bitcast(f32)
        nperi = pkt[:, G_C_NPER:G_C_NPER + 1]
        lsti = pkt[:, G_C_LST:G_C_LST + 1]
        isgi = pkt[:, G_C_ISG:G_C_ISG + 1]
        boni = pkt[:, G_C_BON:G_C_BON + 1]
        oin_i = pkt[:, G_C_OIN:G_C_OIN + SP1]

        # ---- gpsimd ring: iotas, tileB, then the gather chains
        j8i = small.tile([R, S], i32)
        G_.iota(j8i[:], pattern=[[1, S]], base=0, channel_multiplier=0)
        j9i = small.tile([R, SP1], i32)
        G_.iota(j9i[:], pattern=[[1, SP1]], base=0, channel_multiplier=0)
        rev8i = small.tile([R, S], i32)   # 8 - j
        G_.iota(rev8i[:], pattern=[[-1, S]], base=S, channel_multiplier=0)
        iota16 = small.tile([R, 16], i32)
        G_.iota(iota16[:], pattern=[[1, 16]], base=0, channel_multiplier=0)
        qoff0 = small.tile([R, 1], i32)   # r * G_NWIN
        G_.iota(qoff0[:], pattern=[[0, 1]], base=0, channel_multiplier=G_NWIN)
        woffs0 = small.tile([128, 1], i32)  # p * G_NCH
        G_.iota(woffs0[:], pattern=[[0, 1]], base=0, channel_multiplier=G_NCH)
        tlB = stream.tile([128, G_TW], f32, tag="tlB", name="tlB")
        G_.dma_start(tlB[:], tgt_v[:, G_TW:2 * G_TW])

        # ---- ACT casts of smalls
        tokf = small.tile([R, S], f32)
        acast(tokf[:], tok88i)
        nperf = small.tile([R, 1], f32)
        acast(nperf[:], nperi)
        lstf = small.tile([R, 1], f32)
        acast(lstf[:], lsti)
        isgf = small.tile([R, 1], f32)
        acast(isgf[:], isgi)
        bonf = small.tile([R, 1], f32)
        acast(bonf[:], boni)
        j8f = small.tile([R, S], f32)
        acast(j8f[:], j8i[:])
        j9f = small.tile([R, SP1], f32)
        acast(j9f[:], j9i[:])
        rev8f = small.tile([R, S], f32)
        acast(rev8f[:], rev8i[:])

        # ---- dp/tp gather offsets (DVE, f32-exact) -> [64,1]
        lr88f = small.tile([R, S], f32)
        V_.tensor_scalar(lr88f[:], j8f[:], lstf[:, 0:1], None, op0=Alu.add)
        ts(lr88f[:], lr88f[:], float(ROWS - 1), Alu.min, 0.0, Alu.max)
        offs88f = small.tile([R, S], f32)
        ts(offs88f[:], lr88f[:], float(V), Alu.mult)
        tt(offs88f[:], offs88f[:], tokf[:], Alu.add)
        offs88 = small.tile([R, S], i32)
        V_.tensor_copy(out=offs88[:], in_=offs88f[:])
        offs64 = small.tile([ROWS, 1], i32)
        G_.dma_start(offs64[:], offs88[:])
        dp64 = small.tile([ROWS, 1], f32)
        G_.indirect_dma_start(
            out=dp64[:], out_offset=None,
            in_=drf.rearrange("a b -> (a b)").unsqueeze(-1),
            in_offset=bass.IndirectOffsetOnAxis(ap=offs64[:], axis=0))
        tp64 = small.tile([ROWS, 1], f32)
        G_.indirect_dma_start(
            out=tp64[:], out_offset=None,
            in_=tgt.rearrange("a b -> (a b)").unsqueeze(-1),
            in_offset=bass.IndirectOffsetOnAxis(ap=offs64[:], axis=0))
        dp = small.tile([R, S], f32)
        G_.dma_start(dp[:], dp64[:])
        tp = small.tile([R, S], f32)
        G_.dma_start(tp[:], tp64[:])

        # ---- reciprocal of q (approx fast + one Newton step), early on DVE
        y0 = segp.tile([128, G_SEGW], f32)
        V_.reciprocal_approx_fast(out=y0[:], in_=q_seg[:])
        rq = y0
        V_._custom_dve(RECIPROCAL_APPROX_NR, out=rq[:], in0=q_seg[:],
                       in1=y0[:], s0=2.0)

        if dbg is not None:
            dbt = small.tile([R, 4 * S], f32)
            V_.tensor_copy(out=dbt[:, 0:S], in_=dp[:])
            V_.tensor_copy(out=dbt[:, S:2*S], in_=tp[:])
            V_.tensor_copy(out=dbt[:, 2*S:3*S], in_=u88)
            V_.tensor_copy(out=dbt[:, 3*S:4*S], in_=offs88f[:])
            nc.sync.dma_start(dbg[:, :], dbt[:])

        # ---- accept/reject logic (DVE smalls)
        rdp = small.tile([R, S], f32)
        V_.reciprocal(rdp[:], dp[:])
        rt = small.tile([R, S], f32)
        tt(rt[:], tp[:], rdp[:], Alu.mult)
        c2 = small.tile([R, S], f32)
        tt(c2[:], rt[:], u88, Alu.is_ge)
        acc = small.tile([R, S], f32)
        stt(acc[:], dp[:], 0.0, Alu.is_gt, c2[:], Alu.mult)
        validj = small.tile([R, S], f32)
        ts(validj[:], j8f[:], nperf[:, 0:1], Alu.is_lt)
        rej = small.tile([R, S], f32)
        stt(rej[:], acc[:], 0.0, Alu.is_equal, validj[:], Alu.mult)
        rejrev = small.tile([R, S], f32)
        tt(rejrev[:], rej[:], rev8f[:], Alu.mult)
        mrev = small.tile([R, 1], f32)
        V_.tensor_reduce(mrev[:], rejrev[:], axis=AX.X, op=Alu.max)
        first_rej = small.tile([R, 1], f32)   # 8 - mrev (8 when none)
        ts(first_rej[:], mrev[:], -1.0, Alu.mult, float(S), Alu.add)

        # ---- nstar + seg offsets
        nsf = small.tile([R, 1], f32)
        tt(nsf[:], lstf[:], first_rej[:], Alu.add)
        ts(nsf[:], nsf[:], float(ROWS - 1), Alu.min, 0.0, Alu.max)
        nstar_i = small.tile([R, 1], i32)
        acast(nstar_i[:], nsf[:])
        nst16f = small.tile([R, 1], f32)
        ts(nst16f[:], nsf[:], 16.0, Alu.mult)
        iota16f = small.tile([R, 16], f32)
        acast(iota16f[:], iota16[:])
        soff16f = small.tile([R, 16], f32)
        V_.tensor_scalar(soff16f[:], iota16f[:], nst16f[:, 0:1], None, op0=Alu.add)
        soff16 = small.tile([R, 16], i32)
        V_.tensor_copy(out=soff16[:], in_=soff16f[:])
        soffs = small.tile([128, 1], i32)
        G_.dma_start(soffs[:], soff16[:])
        t_seg = segp.tile([128, G_SEGW], f32)
        G_.indirect_dma_start(
            out=t_seg[:], out_offset=None,
            in_=tgt.rearrange("a (b c) -> (a b) c", c=G_SEGW),
            in_offset=bass.IndirectOffsetOnAxis(ap=soffs[:], axis=0))
        d_seg = segp.tile([128, G_SEGW], f32)
        G_.indirect_dma_start(
            out=d_seg[:], out_offset=None,
            in_=drf.rearrange("a (b c) -> (a b) c", c=G_SEGW),
            in_offset=bass.IndirectOffsetOnAxis(ap=soffs[:], axis=0))

        # ---- DVE heavy: MAXB, then ratio sub/mul/reduce, then MAXA
        cmax = small.tile([128, G_NCH], f32)
        V_.tensor_reduce(
            cmax[:, G_TW // G_CW:2 * (G_TW // G_CW)],
            tlB[:].rearrange("p (b c) -> p b c", c=G_CW),
            axis=AX.X, op=Alu.max)
        diff = t_seg
        tt(diff[:], t_seg[:], d_seg[:], Alu.subtract)
        ratio = d_seg
        tt(ratio[:], diff[:], rq[:], Alu.mult)
        rcm = small.tile([128, 16], f32)
        V_.tensor_reduce(rcm[:], ratio[:].rearrange("p (b c) -> p b c", c=G_CW),
                         axis=AX.X, op=Alu.max)
        rcm_req = small.tile([R, 16 * 16], f32)
        G_.dma_start(rcm_req[:], rcm[:])
        V_.tensor_reduce(
            cmax[:, 0:G_TW // G_CW],
            tlA[:].rearrange("p (b c) -> p b c", c=G_CW),
            axis=AX.X, op=Alu.max)

        # ---- ratio argmax finish
        f8 = small.tile([R, 8], f32)
        V_.max(out=f8[:], in_=rcm_req[:])
        fi8 = small.tile([R, 8], u32)
        V_.max_index(out=fi8[:], in_max=f8[:], in_values=rcm_req[:])
        fc_i = small.tile([R, 1], i32)
        acast(fc_i[:], fi8[:, 0:1])
        fcf = small.tile([R, 1], f32)
        acast(fcf[:], fc_i[:])
        woff_t = small.tile([R, 1], i32)
        ts(woff_t[:], nstar_i[:], G_NWIN, Alu.mult)
        tt(woff_t[:], woff_t[:], fc_i[:], Alu.add)
        qoff = small.tile([R, 1], i32)
        tt(qoff[:], qoff0[:], fc_i[:], Alu.add)
        t_win = small.tile([R, G_CW], f32)
        G_.indirect_dma_start(
            out=t_win[:], out_offset=None,
            in_=tgt.rearrange("a (b c) -> (a b) c", c=G_CW),
            in_offset=bass.IndirectOffsetOnAxis(ap=woff_t[:], axis=0))
        d_win = small.tile([R, G_CW], f32)
        G_.indirect_dma_start(
            out=d_win[:], out_offset=None,
            in_=drf.rearrange("a (b c) -> (a b) c", c=G_CW),
            in_offset=bass.IndirectOffsetOnAxis(ap=woff_t[:], axis=0))
        q_win = small.tile([R, G_CW], f32)
        G_.indirect_dma_start(
            out=q_win[:], out_offset=None,
            in_=q.rearrange("a (b c) -> (a b) c", c=G_CW),
            in_offset=bass.IndirectOffsetOnAxis(ap=qoff[:], axis=0))

        # ---- target argmax finish (DVE while window gathers fly)
        hm = small.tile([128, 1], f32)
        V_.tensor_reduce(hm[:], cmax[:], axis=AX.X, op=Alu.max)
        c8 = small.tile([128, 8], f32)
        V_.max(out=c8[:], in_=cmax[:])
        ci8 = small.tile([128, 8], u32)
        V_.max_index(out=ci8[:], in_max=c8[:], in_values=cmax[:])
        ci_i = small.tile([128, 1], i32)
        acast(ci_i[:], ci8[:, 0:1])
        cif = small.tile([128, 1], f32)
        acast(cif[:], ci_i[:])
        woffs = small.tile([128, 1], i32)
        tt(woffs[:], woffs0[:], ci_i[:], Alu.add)
        wt = small.tile([128, G_CW], f32)
        G_.indirect_dma_start(
            out=wt[:], out_offset=None,
            in_=tgt.rearrange("a (b c) -> (a b) c", c=G_CW),
            in_offset=bass.IndirectOffsetOnAxis(ap=woffs[:], axis=0))

        # ---- ratio window recompute (same instruction kinds => same bits)
        y0w = small.tile([R, G_CW], f32)
        y0ws = small.tile([R, G_CW], f32)
        V_.reciprocal_approx_fast(out=y0ws[:], in_=q_win[:])
        V_._custom_dve(RECIPROCAL_APPROX_NR, out=y0w[:], in0=q_win[:],
                       in1=y0ws[:], s0=2.0)
        tt(t_win[:], t_win[:], d_win[:], Alu.subtract)
        tt(t_win[:], t_win[:], y0w[:], Alu.mult)
        w8r = small.tile([R, 8], f32)
        V_.max(out=w8r[:], in_=t_win[:])
        wi8r = small.tile([R, 8], u32)
        V_.max_index(out=wi8r[:], in_max=w8r[:], in_values=t_win[:])
        wrf = small.tile([R, 1], f32)
        acast(wrf[:], wi8r[:, 0:1])
        rec = small.tile([R, 1], f32)
        ts(rec[:], fcf[:], float(G_CW), Alu.mult)
        tt(rec[:], rec[:], wrf[:], Alu.add)

        # ---- target window finish
        w8 = small.tile([128, 8], f32)
        V_.max(out=w8[:], in_=wt[:])
        wi8 = small.tile([128, 8], u32)
        V_.max_index(out=wi8[:], in_max=w8[:], in_values=wt[:])
        wif = small.tile([128, 1], f32)
        acast(wif[:], wi8[:, 0:1])
        halfam = small.tile([128, 1], f32)
        ts(halfam[:], cif[:], float(G_CW), Alu.mult)
        tt(halfam[:], halfam[:], wif[:], Alu.add)
        pk2 = small.tile([128, 2], f32)
        V_.tensor_copy(out=pk2[:, 0:1], in_=hm[:])
        V_.tensor_copy(out=pk2[:, 1:2], in_=halfam[:])
        comb = small.tile([ROWS, 4], f32)   # (lo_m, lo_am, hi_m, hi_am)
        G_.dma_start(comb[:], pk2[:])
        win_hi = small.tile([ROWS, 1], i32)
        tt(win_hi[:], comb[:, 2:3], comb[:, 0:1], Alu.is_gt)
        am_hi = small.tile([ROWS, 1], f32)
        ts(am_hi[:], comb[:, 3:4], float(G_HALF), Alu.add)
        am64 = small.tile([ROWS, 1], f32)
        V_.tensor_copy(out=am64[:], in_=comb[:, 1:2])
        V_.copy_predicated(am64[:], win_hi[:], am_hi[:])
        tgt_am = small.tile([R, S], f32)
        G_.dma_start(tgt_am[:], am64[:])

        # ---- greedy logic
        mism = small.tile([R, S], f32)
        tt(mism[:], tokf[:], tgt_am[:], Alu.not_equal)
        tt(mism[:], mism[:], validj[:], Alu.mult)
        mmrev = small.tile([R, S], f32)
        tt(mmrev[:], mism[:], rev8f[:], Alu.mult)
        mm_max = small.tile([R, 1], f32)
        V_.tensor_reduce(mm_max[:], mmrev[:], axis=AX.X, op=Alu.max)
        first_mm = small.tile([R, 1], f32)
        ts(first_mm[:], mm_max[:], -1.0, Alu.mult, float(S), Alu.add)
        copy_len = small.tile([R, 1], f32)
        ts(copy_len[:], first_mm[:], 1.0, Alu.add)
        tt(copy_len[:], copy_len[:], nperf[:], Alu.min)

        # ---- output assembly
        draft9 = small.tile([R, SP1], f32)
        V_.memset(draft9[:, S:SP1], 0.0)
        V_.tensor_copy(out=draft9[:, 0:S], in_=tokf[:])
        am9 = small.tile([R, SP1], f32)
        V_.memset(am9[:, S:SP1], 0.0)
        V_.tensor_copy(out=am9[:, 0:S], in_=tgt_am[:])
        validj9 = small.tile([R, SP1], f32)
        ts(validj9[:], j9f[:], nperf[:, 0:1], Alu.is_lt)
        jeqn = small.tile([R, SP1], f32)
        ts(jeqn[:], j9f[:], nperf[:, 0:1], Alu.is_equal)
        on = small.tile([R, SP1], f32)
        acast(on[:], oin_i)
        dmask = small.tile([R, SP1], i32)
        stt(dmask[:], j9f[:], first_rej[:, 0:1], Alu.is_lt, validj9[:], Alu.mult)
        V_.copy_predicated(on[:], dmask[:], draft9[:])
        rmask = small.tile([R, SP1], i32)
        stt(rmask[:], j9f[:], first_rej[:, 0:1], Alu.is_equal, validj9[:], Alu.mult)
        V_.copy_predicated(on[:], rmask[:], rec[:].to_broadcast([R, SP1]))
        bn = small.tile([R, 1], f32)
        tt(bn[:], first_rej[:], nperf[:], Alu.is_ge)
        bmn = small.tile([R, SP1], i32)
        tt(bmn[:], jeqn[:], bn[:].to_broadcast([R, SP1]), Alu.mult)
        V_.copy_predicated(on[:], bmn[:], bonf[:].to_broadcast([R, SP1]))
        og = small.tile([R, SP1], f32)
        acast(og[:], oin_i)
        clm = small.tile([R, SP1], i32)
        ts(clm[:], j9f[:], copy_len[:, 0:1], Alu.is_lt)
        V_.copy_predicated(og[:], clm[:], am9[:])
        bg = small.tile([R, 1], f32)
        tt(bg[:], first_mm[:], nperf[:], Alu.is_ge)
        bmg = small.tile([R, SP1], i32)
        tt(bmg[:], jeqn[:], bg[:].to_broadcast([R, SP1]), Alu.mult)
        V_.copy_predicated(og[:], bmg[:], bonf[:].to_broadcast([R, SP1]))
        isg9 = small.tile([R, SP1], i32)
        V_.tensor_copy(out=isg9[:], in_=isgi.to_broadcast([R, SP1]))
        V_.copy_predicated(on[:], isg9[:], og[:])
        outi = small.tile([R, SP1], i32)
        V_.tensor_copy(out=outi[:], in_=on[:])
        nc.sync.dma_start(out[:, :], outi[:])





def shard_inputs_gen(inputs, n_cores=8):
    cu = inputs["cu_num_draft_tokens"].astype(np.int64)
    B = cu.shape[0]
    N = inputs["draft_token_ids"].shape[0]
    n_per = np.diff(np.concatenate([[0], cu]))
    start = cu - n_per
    Rc = B // n_cores
    gidx = np.clip(start[:, None] + np.arange(S)[None, :], 0, N - 1)
    in_maps = []
    for c in range(n_cores):
        rs = slice(c * Rc, (c + 1) * Rc)
        row0 = int(start[c * Rc])
        idx = np.arange(row0, row0 + ROWS)
        if idx[-1] < N:
            tgt_c = inputs["target_probs"][row0:row0 + ROWS]
            drf_c = inputs["draft_probs"][row0:row0 + ROWS]
        else:
            idxc = np.clip(idx, 0, N - 1)
            tgt_c = inputs["target_probs"][idxc]
            drf_c = inputs["draft_probs"][idxc]
        g = gidx[rs]
        pkc = np.zeros((R, G_PKW), np.int32)
        pkc[:, G_C_TOK:G_C_TOK + 8] = inputs["draft_token_ids"][g].astype(np.int32)
        pkc[:, G_C_U:G_C_U + 8] = (
            inputs["uniform_probs"][g].astype(np.float32).view(np.int32))
        pkc[:, G_C_NPER] = n_per[rs].astype(np.int32)
        pkc[:, G_C_LST] = (start[rs] - row0).astype(np.int32)
        pkc[:, G_C_ISG] = inputs["is_greedy"][rs].astype(np.int32)
        pkc[:, G_C_BON] = inputs["bonus_token_ids"][rs].astype(np.int32)
        pkc[:, G_C_OIN:G_C_OIN + SP1] = inputs["output_token_ids"][rs].astype(np.int32)
        in_maps.append(dict(
            tgt=np.ascontiguousarray(tgt_c, dtype=np.float32),
            drf=np.ascontiguousarray(drf_c, dtype=np.float32),
            q=np.ascontiguousarray(inputs["q"][rs], dtype=np.float32),
            pk=pkc,
        ))
    return in_maps


def assemble_outputs_gen(results):
    return np.concatenate([r["out"] for r in results], axis=0).astype(np.int32)


# ---------------- dispatch ----------------

_CACHE = {}


def _get_nc(kind):
    if kind not in _CACHE:
        if kind == "fast":
            _CACHE[kind] = build_kernel_fast(n_devices=8)
        else:
            _CACHE[kind] = build_kernel_gen(n_devices=8)
    return _CACHE[kind]


def _kernel_numpy(output_token_ids, cu_num_draft_tokens, draft_token_ids,
                  draft_probs, target_probs, bonus_token_ids, uniform_probs,
                  q, is_greedy):
    """Shape-agnostic reference fallback (host compute; only used for inputs
    the compiled device programs cannot fit)."""
    out = np.array(output_token_ids, dtype=np.int32).copy()
    Bb, Sp1 = out.shape
    Sl = Sp1 - 1
    Nt = draft_token_ids.shape[0]
    cu = np.asarray(cu_num_draft_tokens, dtype=np.int64)
    n_per = np.diff(np.concatenate([[0], cu]))
    start_ = cu - n_per
    tam = target_probs.argmax(axis=-1).astype(np.int32)
    prob = np.maximum(target_probs - draft_probs, 0.0)
    req_id = np.searchsorted(cu, np.arange(Nt), side="right")
    rec = (prob / q[req_id]).argmax(axis=1).astype(np.int32)
    for r in range(Bb):
        npr = int(n_per[r]); st = int(start_[r])
        if is_greedy[r]:
            k = npr
            for j in range(npr):
                g = min(st + j, Nt - 1)
                if draft_token_ids[g] != tam[g]:
                    k = j
                    break
            for j in range(min(k + 1, npr)):
                out[r, j] = tam[min(st + j, Nt - 1)]
            if k >= npr and npr < Sp1:
                out[r, npr] = bonus_token_ids[r]
        else:
            fr = Sl
            for j in range(npr):
                g = min(st + j, Nt - 1)
                dp = draft_probs[g, draft_token_ids[g]]
                tp = target_probs[g, draft_token_ids[g]]
                ok = dp > 0 and (tp / dp) >= uniform_probs[g]
                if not ok:
                    fr = j
                    break
            for j in range(npr):
                g = min(st + j, Nt - 1)
                if j < fr:
                    out[r, j] = draft_token_ids[g]
                elif j == fr:
                    out[r, j] = rec[g]
                else:
                    break
            if fr >= npr and npr < Sp1:
                out[r, npr] = bonus_token_ids[r]
    return out


def _shapes_ok(inputs):
    try:
        return (inputs["output_token_ids"].shape == (64, 9)
                and inputs["cu_num_draft_tokens"].shape == (64,)
                and inputs["draft_token_ids"].shape == (512,)
                and inputs["draft_probs"].shape == (512, 32000)
                and inputs["target_probs"].shape == (512, 32000)
                and inputs["bonus_token_ids"].shape == (64,)
                and inputs["uniform_probs"].shape == (512,)
                and inputs["q"].shape == (64, 32000)
                and inputs["is_greedy"].shape == (64,))
    except Exception:
        return False


def kernel(**inputs):
    inputs = {k: np.asarray(v) for k, v in inputs.items()}
    if not _shapes_ok(inputs):
        return _kernel_numpy(**inputs)
    cu = inputs["cu_num_draft_tokens"].astype(np.int64)
    n_per = np.diff(np.concatenate([[0], cu]))
    uniform = bool((n_per == S).all())
    perm = plan_permutation(inputs) if uniform else None
    if perm is not None:
        nc = _get_nc("fast")
        in_maps = shard_inputs_fast(inputs, perm)
        res = bass_utils.run_bass_kernel_spmd(nc, in_maps,
                                              core_ids=list(range(8)))
        return assemble_outputs_fast(res.results, perm)
    if bool((n_per >= 0).all()) and bool((n_per <= S).all()):
        nc = _get_nc("gen")
        in_maps = shard_inputs_gen(inputs)
        res = bass_utils.run_bass_kernel_spmd(nc, in_maps,
                                              core_ids=list(range(8)))
        return assemble_outputs_gen(res.results)
    return _kernel_numpy(**inputs)

